# revision 14
# baseline (speedup 1.0000x reference)
"""Multi-head attention (B=4, S=2048, D=1024, H=16) on 8 TRN2 NeuronCores.

Sharding: core c -> (batch b = c//2, head-group g = c%2 of 8 heads).
Data parallel over batch, tensor parallel over heads; each core computes
its group's QKV projection slices, causal attention for its 8 heads, and
the partial output projection. Host sums the two per-batch partials
(the tensor-parallel unshard) and adds the V-bias epilogue.

On-device layout is "features on partitions": x, Q, K arrive/stay
transposed [feat, seq]; attention scores are computed directly in
transposed form S.T[k, q] so the exp'd probabilities feed the PV matmul
without any on-chip transpose. The softmax denominator rides the PV
matmul as an appended ones-column of V; normalization is a fast
reciprocal + K=1 broadcast matmul + DVE multiply.

Scheduling: TensorE executes in program order, so the QKV projection for
seq block sb+1 and the output projection for q-tile qi-1 are emitted as
small 2-PSUM-bank pieces woven between attention blocks of q-tile qi.
That keeps the PE busy while the (otherwise pacing) softmax EXP runs on
ScalarE, instead of running projection and attention as serial phases.
The first seq block's projection uses a transient 8-bank PSUM layout so
the first matmul only waits for one x/w chunk pair.
"""

import os
import numpy as np

B, S, D, H = 4, 2048, 1024, 16
DK = D // H          # 64
HPC = H // 2         # heads per core = 8
GD = HPC * DK        # group feature width = 512
QT = 512             # q-tile width (free dim of S.T chunks)
KTL = 128            # k-tile length (partition dim of S.T chunks)
N_QT = S // QT       # 4
N_KT = S // KTL      # 16
SB = 512             # projection seq block
STRW = 256           # padded width of multiplicative mask strips
NEG = np.float32(-1e9)
SCALE = 1.0 / np.sqrt(np.float32(DK))

_cache = {}
last_results = None


def _classify_mask(mask2d):
    """Classify each (q-tile, k-tile) block of the [S,S] bool mask.

    Returns (plan, strips, biases):
      plan[qi] = list over valid kt of (kt, kind, a, b):
        kind 0 = clean (no masking)
        kind 1 = staircase: a = q0 (suffix start), b = (strip_idx, strip_w)
        kind 2 = general:   a = bias_idx
      strips: list of [KTL, STRW] f32 0/1 multiplicative masks (padded)
      biases: list of [KTL, QT] f32 additive -1e9/0 masks
    Blocks are in S.T (k, q) layout.
    """
    kl = np.arange(KTL)[:, None]
    ql = np.arange(QT)[None, :]
    plan = []
    strips, strip_keys = [], {}
    biases, bias_keys = [], {}
    for qi in range(N_QT):
        row = []
        for kt in range(N_KT):
            blk = mask2d[qi * QT:(qi + 1) * QT, kt * KTL:(kt + 1) * KTL].T
            if blk.all():
                continue
            if not blk.any():
                row.append((kt, 0, 0, None))
                continue
            dj = kt * KTL - qi * QT
            stair = (0 <= dj <= QT - KTL and np.array_equal(blk, kl + dj > ql)
                     and not os.environ.get("KERNEL_NO_STAIR"))
            q0 = min(dj, QT - STRW) if stair else 0
            if stair and (q0 == 0 or row):
                w = min(dj + KTL, QT) - q0
                pat = (~blk[:, q0:q0 + w]).astype(np.float32)
                key = (w, pat.tobytes())
                if key not in strip_keys:
                    strip_keys[key] = len(strips)
                    p = np.zeros((KTL, STRW), np.float32)
                    p[:, :w] = pat
                    strips.append(p)
                row.append((kt, 1, q0, (strip_keys[key], w)))
            else:
                bias = np.where(blk, NEG, np.float32(0.0))
                key = bias.tobytes()
                if key not in bias_keys:
                    bias_keys[key] = len(biases)
                    biases.append(bias)
                row.append((kt, 2, bias_keys[key], None))
        if not row:
            # fully-masked q-row: include everything with full bias so the
            # softmax matches the reference's uniform distribution.
            bias = np.full((KTL, QT), NEG, np.float32)
            key = bias.tobytes()
            if key not in bias_keys:
                bias_keys[key] = len(biases)
                biases.append(bias)
            row = [(kt, 2, bias_keys[key], None) for kt in range(N_KT)]
        plan.append(row)
    return plan, strips, biases


def _build(plan, n_strips, n_biases):
    import concourse.bacc as bacc
    import concourse.tile as tile
    import concourse.mybir as mybir
    from contextlib import ExitStack

    f32 = mybir.dt.float32
    f16 = mybir.dt.float16
    Exp = mybir.ActivationFunctionType.Exp

    nc = bacc.Bacc(trn_type="TRN2", target_bir_lowering=False, debug=False)
    xT = nc.dram_tensor("xT", [D, S], f16, kind="ExternalInput").ap()
    w_qk = nc.dram_tensor("w_qk", [D, 2 * GD], f16, kind="ExternalInput").ap()
    b_qk = nc.dram_tensor("b_qk", [2 * GD], f32, kind="ExternalInput").ap()
    w_v = nc.dram_tensor("w_v", [D, GD], f16, kind="ExternalInput").ap()
    wo_T = nc.dram_tensor("wo_T", [GD, D], f16, kind="ExternalInput").ap()
    trid = nc.dram_tensor("trid", [KTL, KTL], f16, kind="ExternalInput").ap()
    outT = nc.dram_tensor("outT", [D, S], f16, kind="ExternalOutput").ap()

    ND = D // 128       # 8 contraction chunks
    NM = 2 * GD // 128  # 8 QK feature chunks (0-3 = Q.T, 4-7 = K.T)
    NK3 = GD // 128     # 4 output-projection contraction chunks
    NSB = S // SB       # 4

    with tile.TileContext(nc) as tc, ExitStack() as ctx:
        singles = ctx.enter_context(tc.tile_pool(name="singles", bufs=1))
        qkt_pool = ctx.enter_context(tc.tile_pool(name="qkt", bufs=1))
        v_pool = ctx.enter_context(tc.tile_pool(name="vp", bufs=1))
        otq_pool = ctx.enter_context(tc.tile_pool(name="otq", bufs=2))
        xs_pool = ctx.enter_context(tc.tile_pool(name="p1x", bufs=2))

        qkt = [qkt_pool.tile([128, S], f16, tag=f"qkt{m}", name=f"qkt{m}")
               for m in range(NM)]
        ones_col = singles.tile([1, DK], f16)
        v_sb = [v_pool.tile([128, HPC, 128], f16, tag=f"v{t}", name=f"v{t}")
                for t in range(N_KT)]
        bqk_t = singles.tile([128, NM], f32)
        wqk_t = [singles.tile([128, 2 * GD], f16, tag=f"wqk{k}", name=f"wqk{k}")
                 for k in range(ND)]
        wv_t = [singles.tile([128, GD], f16, tag=f"wv{k}", name=f"wv{k}")
                for k in range(ND)]
        wo_t = [singles.tile([128, D], f16, tag=f"wo{k}", name=f"wo{k}")
                for k in range(NK3)]
        tri = singles.tile([KTL, KTL], f16)

        # Load order matters: the first seq block's projection contracts
        # k-outer, so (xs[k], wqk[k]) pairs in k order come first; wv/wo/
        # masks follow and finish while the QK matmuls run.
        xs_cur = [xs_pool.tile([128, SB], f16, tag=f"x{k}", name=f"xs{k}")
                  for k in range(ND)]
        for k in range(ND):
            nc.sync.dma_start(out=xs_cur[k], in_=xT[128 * k:128 * (k + 1), 0:SB])
            nc.sync.dma_start(out=wqk_t[k], in_=w_qk[128 * k:128 * (k + 1)])
        nc.sync.dma_start(out=bqk_t, in_=b_qk.rearrange("(m p) -> p m", p=128))
        for k in range(ND):
            nc.sync.dma_start(out=wv_t[k], in_=w_v[128 * k:128 * (k + 1)])
        for k in range(NK3):
            nc.sync.dma_start(out=wo_t[k], in_=wo_T[128 * k:128 * (k + 1)])
        nc.sync.dma_start(out=tri, in_=trid)
        nc.vector.memset(ones_col, 1.0)
        for t in range(N_KT):
            nc.vector.memset(v_sb[t][:, :, 0:1], 1.0)
            nc.vector.memset(v_sb[t][:, :, 1:DK], 0.0)

        # ==== projection for seq block 0: transient 8-bank k-outer layout
        # (first matmul only waits for xs[0] + wqk[0]) ====
        with tc.tile_pool(name="p1ps0", bufs=8, space="PSUM") as p1ps0:
            pss = [p1ps0.tile([128, SB], f32, tag="p10", name=f"ps0{m}")
                   for m in range(NM)]
            for k in range(ND):
                for m in range(NM):
                    nc.tensor.matmul(
                        pss[m][:], wqk_t[k][:, 128 * m:128 * (m + 1)],
                        xs_cur[k][:], start=(k == 0), stop=(k == ND - 1))
            for m in range(NM):
                nc.vector.tensor_scalar_add(qkt[m][:, 0:SB], pss[m][:],
                                            bqk_t[:, m:m + 1])
            for tt in range(SB // 128):
                ps = p1ps0.tile([128, GD], f32, tag="p10", name="ps0_v")
                for k in range(ND):
                    nc.tensor.matmul(
                        ps[:], xs_cur[k][:, 128 * tt:128 * (tt + 1)], wv_t[k][:],
                        start=(k == 0), stop=(k == ND - 1))
                nc.vector.tensor_copy(
                    out=v_sb[tt][:, :, DK:2 * DK],
                    in_=ps[:].rearrange("p (h d) -> p h d", h=HPC))

        # ==== steady-state pools: 4 + 2 + 2 = 8 PSUM banks ====
        st_pool = ctx.enter_context(tc.tile_pool(name="st", bufs=2, space="PSUM"))
        ot_pool = ctx.enter_context(tc.tile_pool(name="ot", bufs=2, space="PSUM"))
        p1p = ctx.enter_context(tc.tile_pool(name="p1p", bufs=2, space="PSUM"))
        pt_pool = ctx.enter_context(tc.tile_pool(name="pt", bufs=6))
        rr_pool = ctx.enter_context(tc.tile_pool(name="rr", bufs=4))
        p3o = ctx.enter_context(tc.tile_pool(name="p3o", bufs=4))

        xs_next = [None]

        def p1_pieces(sb):
            """Projection for seq block sb as 12 single-PSUM-chunk pieces
            (m-outer, k-inner contraction into 1 bank at a time)."""
            xs = xs_next[0]

            def qk_piece(m=0, xs=xs, sb=sb):
                ps = p1p.tile([128, SB], f32, tag="p1", name="ps_qk")
                for k in range(ND):
                    nc.tensor.matmul(
                        ps[:], wqk_t[k][:, 128 * m:128 * (m + 1)], xs[k][:],
                        start=(k == 0), stop=(k == ND - 1))
                nc.vector.tensor_scalar_add(
                    qkt[m][:, SB * sb:SB * (sb + 1)], ps[:], bqk_t[:, m:m + 1])

            def v_piece(tt=0, xs=xs, sb=sb):
                t = sb * (SB // 128) + tt
                ps = p1p.tile([128, GD], f32, tag="p1", name="ps_v")
                for k in range(ND):
                    nc.tensor.matmul(
                        ps[:], xs[k][:, 128 * tt:128 * (tt + 1)], wv_t[k][:],
                        start=(k == 0), stop=(k == ND - 1))
                nc.vector.tensor_copy(
                    out=v_sb[t][:, :, DK:2 * DK],
                    in_=ps[:].rearrange("p (h d) -> p h d", h=HPC))

            for m in range(NM):
                yield (lambda m=m: qk_piece(m))
            for tt in range(SB // 128):
                yield (lambda tt=tt: v_piece(tt))

        def p3_pieces(qi, otq):
            """Output projection for q-tile qi as 8 single-chunk pieces."""
            def piece(m=0, qi=qi, otq=otq):
                ps = p1p.tile([128, QT], f32, tag="p1", name="ps_o")
                for k in range(NK3):
                    nc.tensor.matmul(
                        ps[:], wo_t[k][:, 128 * m:128 * (m + 1)], otq[k][:],
                        start=(k == 0), stop=(k == NK3 - 1))
                ob = p3o.tile([128, QT], f16, tag="ob", name="ob")
                nc.vector.tensor_copy(out=ob[:], in_=ps[:])
                nc.sync.dma_start(
                    out=outT[128 * m:128 * (m + 1), QT * qi:QT * (qi + 1)],
                    in_=ob[:])
            for m in range(D // 128):
                yield (lambda m=m: piece(m))

        # ==== attention q-tiles with woven projection pieces ====
        prev_otq = None
        for qi in range(N_QT):
            fill = []
            if qi + 1 < NSB:
                xs_next[0] = [xs_pool.tile([128, SB], f16, tag=f"x{k}",
                                           name=f"xs{k}") for k in range(ND)]
                for k in range(ND):
                    nc.sync.dma_start(
                        out=xs_next[0][k],
                        in_=xT[128 * k:128 * (k + 1),
                               SB * (qi + 1):SB * (qi + 2)])
                fill.extend(p1_pieces(qi + 1))
            if prev_otq is not None:
                fill.extend(p3_pieces(qi - 1, prev_otq))

            n_kts = 4 * (qi + 1)
            n_blocks = max(n_kts * (HPC // 2), 1)
            pace = len(fill) / n_blocks
            acc = 0.0
            fi = 0

            otq = [otq_pool.tile([128, QT], f16, tag=f"otq{m}", name=f"otq{m}")
                   for m in range(NK3)]
            for hp in range(HPC // 2):
                ot_ps = [ot_pool.tile([128, QT], f32, tag="ot", name="ot_ps")
                         for _ in range(2)]
                for kt in range(n_kts):
                    dj = 128 * (kt - (n_kts - 4))
                    q0 = max(dj, 0)
                    st = st_pool.tile([128, 2, QT], f32, tag="st", name="st")
                    for h in range(2):
                        lo, hi = 64 * h, 64 * h + 64
                        nc.tensor.matmul(
                            st[:, h, q0:QT],
                            qkt[4 + hp][lo:hi, KTL * kt:KTL * (kt + 1)],
                            qkt[hp][lo:hi, QT * qi + q0:QT * (qi + 1)],
                            start=True, stop=True, tile_position=(64 * h, 0))
                    # weave projection pieces while EXP runs on ScalarE
                    acc += pace
                    while acc >= 1.0 and fi < len(fill):
                        fill[fi]()
                        fi += 1
                        acc -= 1.0
                    pt = pt_pool.tile([128, 2, QT], f16, tag="pt", name="pt")
                    nc.scalar.activation(out=pt[:, :, q0:QT], in_=st[:, :, q0:QT],
                                         func=Exp, scale=float(SCALE))
                    if dj >= 0:
                        nc.vector.tensor_mul(
                            pt[:, :, dj:dj + 128], pt[:, :, dj:dj + 128],
                            tri[:, None, :].broadcast_to([KTL, 2, KTL]))
                    for h in range(2):
                        nc.tensor.matmul(
                            ot_ps[h][:, q0:QT],
                            v_sb[kt][:, 2 * hp + h, :],
                            pt[:, h, q0:QT],
                            start=(kt == 0), stop=(kt == n_kts - 1))
                # softmax normalization: 1/denom broadcast via K=1 matmul
                r16s = []
                for h in range(2):
                    r_row = rr_pool.tile([1, QT], f32, tag=f"rrow{h}",
                                         name="r_row")
                    nc.vector.reciprocal_approx_fast(out=r_row[:],
                                                     in_=ot_ps[h][0:1, :])
                    r16 = rr_pool.tile([1, QT], f16, tag=f"r16{h}", name="r16")
                    nc.vector.tensor_copy(out=r16[:], in_=r_row[:])
                    r16s.append(r16)
                for h in range(2):
                    rb_ps = p1p.tile([DK, QT], f32, tag="p1", name="rb_ps")
                    nc.tensor.matmul(rb_ps[:], ones_col[:], r16s[h][:],
                                     start=True, stop=True)
                    rb_sb = rr_pool.tile([DK, QT], f32, tag=f"rbsb{h}",
                                         name="rb_sb")
                    nc.vector.tensor_copy(out=rb_sb[:], in_=rb_ps[:])
                    nc.vector.tensor_mul(otq[hp][64 * h:64 * h + 64, :],
                                         ot_ps[h][DK:2 * DK, :], rb_sb[:])
            while fi < len(fill):
                fill[fi]()
                fi += 1
            prev_otq = otq
        # final q-tile's output projection
        for piece in p3_pieces(N_QT - 1, prev_otq):
            piece()
    nc.compile()
    return nc


def kernel(encodings_for_qkv, mask, w_qkv, b_qkv, w_o):
    global last_results
    from concourse.bass_utils import run_bass_kernel_spmd

    x = np.ascontiguousarray(np.asarray(encodings_for_qkv, dtype=np.float32))
    mask2d = np.asarray(mask).reshape(S, S).astype(bool)
    w_qkv = np.asarray(w_qkv, dtype=np.float32)
    b_qkv = np.asarray(b_qkv, dtype=np.float32)
    w_o = np.asarray(w_o, dtype=np.float32)

    causal = np.triu(np.ones((S, S), dtype=bool), k=1)
    assert np.array_equal(mask2d, causal), "kernel specialised for causal mask"
    if "nc" not in _cache:
        _cache["nc"] = _build(None, 0, 0)
    nc = _cache["nc"]

    kl = np.arange(KTL)[:, None]
    jl = np.arange(KTL)[None, :]
    tri = np.where(kl > jl, np.float32(0), np.float32(1)).astype(np.float16)
    wT = np.ascontiguousarray(w_qkv.T)        # [D, 3D]
    woT_full = w_o.T                          # [D(in), D(out)]

    in_maps = []
    for c in range(8):
        b, g = divmod(c, 2)
        cols = slice(GD * g, GD * (g + 1))
        w_qk_g = np.ascontiguousarray(
            np.concatenate([wT[:, 0 * D:][:, cols], wT[:, 1 * D:][:, cols]], axis=1))
        b_qk_g = np.ascontiguousarray(
            np.concatenate([b_qkv[0 * D:1 * D][cols], b_qkv[1 * D:2 * D][cols]]))
        w_v_g = np.ascontiguousarray(wT[:, 2 * D:][:, cols])
        wo_T_g = np.ascontiguousarray(woT_full[cols, :])
        in_maps.append({
            "xT": np.ascontiguousarray(x[b].T).astype(np.float16),
            "w_qk": w_qk_g.astype(np.float16), "b_qk": b_qk_g,
            "w_v": w_v_g.astype(np.float16),
            "wo_T": wo_T_g.astype(np.float16),
            "trid": tri,
        })

    trace = bool(int(os.environ.get("KERNEL_PROFILE", "0")))
    res = run_bass_kernel_spmd(nc, in_maps, core_ids=list(range(8)),
                               trace=trace,
                               trace_cores=list(range(8)) if trace else None)
    last_results = res

    out = np.empty((B, S, D), dtype=np.float32)
    for b in range(B):
        acc = (res.results[2 * b]["outT"].astype(np.float32)
               + res.results[2 * b + 1]["outT"].astype(np.float32))
        out[b] = acc.T
    # V-bias epilogue: softmax rows sum to 1, so the V bias contributes a
    # constant (b_v @ w_o.T) to every sequence position.
    out += (b_qkv[2 * D:] @ woT_full).reshape(1, 1, D)
    return out



# revision 15
# speedup vs baseline: 1.0301x; 1.0301x over previous
"""Multi-head attention (B=4, S=2048, D=1024, H=16) on 8 TRN2 NeuronCores.

Sharding: core c -> (batch b = c//2, head-group g = c%2 of 8 heads).
Data parallel over batch, tensor parallel over heads; each core computes
its group's QKV projection slices, causal attention for its 8 heads, and
the partial output projection. Host sums the two per-batch partials
(the tensor-parallel unshard) and adds the V-bias epilogue.

On-device layout is "features on partitions": x, Q, K arrive/stay
transposed [feat, seq]; attention scores are computed directly in
transposed form S.T[k, q] so the exp'd probabilities feed the PV matmul
without any on-chip transpose. The softmax denominator rides the PV
matmul as an appended ones-column of V; normalization is a fast
reciprocal + K=1 broadcast matmul + DVE multiply.

Scheduling: TensorE executes in program order, so the QKV projection for
seq block sb+1 and the output projection for q-tile qi-1 are emitted as
small 2-PSUM-bank pieces woven between attention blocks of q-tile qi.
That keeps the PE busy while the (otherwise pacing) softmax EXP runs on
ScalarE, instead of running projection and attention as serial phases.
The first seq block's projection uses a transient 8-bank PSUM layout so
the first matmul only waits for one x/w chunk pair.
"""

import os
import numpy as np

B, S, D, H = 4, 2048, 1024, 16
DK = D // H          # 64
HPC = H // 2         # heads per core = 8
GD = HPC * DK        # group feature width = 512
QT = 512             # q-tile width (free dim of S.T chunks)
KTL = 128            # k-tile length (partition dim of S.T chunks)
N_QT = S // QT       # 4
N_KT = S // KTL      # 16
SB = 512             # projection seq block
STRW = 256           # padded width of multiplicative mask strips
NEG = np.float32(-1e9)
SCALE = 1.0 / np.sqrt(np.float32(DK))

_cache = {}
last_results = None


def _classify_mask(mask2d):
    """Classify each (q-tile, k-tile) block of the [S,S] bool mask.

    Returns (plan, strips, biases):
      plan[qi] = list over valid kt of (kt, kind, a, b):
        kind 0 = clean (no masking)
        kind 1 = staircase: a = q0 (suffix start), b = (strip_idx, strip_w)
        kind 2 = general:   a = bias_idx
      strips: list of [KTL, STRW] f32 0/1 multiplicative masks (padded)
      biases: list of [KTL, QT] f32 additive -1e9/0 masks
    Blocks are in S.T (k, q) layout.
    """
    kl = np.arange(KTL)[:, None]
    ql = np.arange(QT)[None, :]
    plan = []
    strips, strip_keys = [], {}
    biases, bias_keys = [], {}
    for qi in range(N_QT):
        row = []
        for kt in range(N_KT):
            blk = mask2d[qi * QT:(qi + 1) * QT, kt * KTL:(kt + 1) * KTL].T
            if blk.all():
                continue
            if not blk.any():
                row.append((kt, 0, 0, None))
                continue
            dj = kt * KTL - qi * QT
            stair = (0 <= dj <= QT - KTL and np.array_equal(blk, kl + dj > ql)
                     and not os.environ.get("KERNEL_NO_STAIR"))
            q0 = min(dj, QT - STRW) if stair else 0
            if stair and (q0 == 0 or row):
                w = min(dj + KTL, QT) - q0
                pat = (~blk[:, q0:q0 + w]).astype(np.float32)
                key = (w, pat.tobytes())
                if key not in strip_keys:
                    strip_keys[key] = len(strips)
                    p = np.zeros((KTL, STRW), np.float32)
                    p[:, :w] = pat
                    strips.append(p)
                row.append((kt, 1, q0, (strip_keys[key], w)))
            else:
                bias = np.where(blk, NEG, np.float32(0.0))
                key = bias.tobytes()
                if key not in bias_keys:
                    bias_keys[key] = len(biases)
                    biases.append(bias)
                row.append((kt, 2, bias_keys[key], None))
        if not row:
            # fully-masked q-row: include everything with full bias so the
            # softmax matches the reference's uniform distribution.
            bias = np.full((KTL, QT), NEG, np.float32)
            key = bias.tobytes()
            if key not in bias_keys:
                bias_keys[key] = len(biases)
                biases.append(bias)
            row = [(kt, 2, bias_keys[key], None) for kt in range(N_KT)]
        plan.append(row)
    return plan, strips, biases


def _build(plan, n_strips, n_biases):
    import concourse.bacc as bacc
    import concourse.tile as tile
    import concourse.mybir as mybir
    from contextlib import ExitStack

    f32 = mybir.dt.float32
    f16 = mybir.dt.float16
    Exp = mybir.ActivationFunctionType.Exp

    nc = bacc.Bacc(trn_type="TRN2", target_bir_lowering=False, debug=False)
    xT = nc.dram_tensor("xT", [D, S], f16, kind="ExternalInput").ap()
    w_qk = nc.dram_tensor("w_qk", [D, 2 * GD], f16, kind="ExternalInput").ap()
    b_qk = nc.dram_tensor("b_qk", [2 * GD], f32, kind="ExternalInput").ap()
    w_v = nc.dram_tensor("w_v", [D, GD], f16, kind="ExternalInput").ap()
    wo_T = nc.dram_tensor("wo_T", [GD, D], f16, kind="ExternalInput").ap()
    maskm = nc.dram_tensor("maskm", [max(n_strips, 1), KTL, STRW], f16,
                           kind="ExternalInput").ap()
    maskb = nc.dram_tensor("maskb", [max(n_biases, 1), KTL, QT], f32,
                           kind="ExternalInput").ap()
    outT = nc.dram_tensor("outT", [D, S], f16, kind="ExternalOutput").ap()

    ND = D // 128       # 8 contraction chunks
    NM = 2 * GD // 128  # 8 QK feature chunks (0-3 = Q.T, 4-7 = K.T)
    NK3 = GD // 128     # 4 output-projection contraction chunks
    NSB = S // SB       # 4

    with tile.TileContext(nc) as tc, ExitStack() as ctx:
        singles = ctx.enter_context(tc.tile_pool(name="singles", bufs=1))
        qkt_pool = ctx.enter_context(tc.tile_pool(name="qkt", bufs=1))
        v_pool = ctx.enter_context(tc.tile_pool(name="vp", bufs=1))
        otq_pool = ctx.enter_context(tc.tile_pool(name="otq", bufs=2))
        xs_pool = ctx.enter_context(tc.tile_pool(name="p1x", bufs=2))

        qkt = [qkt_pool.tile([128, S], f16, tag=f"qkt{m}", name=f"qkt{m}")
               for m in range(NM)]
        ones_col = singles.tile([1, DK], f16)
        v_sb = [v_pool.tile([128, HPC, 128], f16, tag=f"v{t}", name=f"v{t}")
                for t in range(N_KT)]
        bqk_t = singles.tile([128, NM], f32)
        wqk_t = [singles.tile([128, 2 * GD], f16, tag=f"wqk{k}", name=f"wqk{k}")
                 for k in range(ND)]
        wv_t = [singles.tile([128, GD], f16, tag=f"wv{k}", name=f"wv{k}")
                for k in range(ND)]
        wo_t = [singles.tile([128, D], f16, tag=f"wo{k}", name=f"wo{k}")
                for k in range(NK3)]
        mm_t = [singles.tile([KTL, STRW], f16, tag=f"mm{i}", name=f"mm{i}")
                for i in range(n_strips)]
        mb_t = [singles.tile([KTL, QT], f32, tag=f"mb{i}", name=f"mb{i}")
                for i in range(n_biases)]

        # Load order matters: the first seq block's projection contracts
        # k-outer, so (xs[k], wqk[k]) pairs in k order come first; wv/wo/
        # masks follow and finish while the QK matmuls run.
        xs_cur = [xs_pool.tile([128, SB], f16, tag=f"x{k}", name=f"xs{k}")
                  for k in range(ND)]
        for k in range(ND):
            nc.sync.dma_start(out=xs_cur[k], in_=xT[128 * k:128 * (k + 1), 0:SB])
            nc.sync.dma_start(out=wqk_t[k], in_=w_qk[128 * k:128 * (k + 1)])
        nc.sync.dma_start(out=bqk_t, in_=b_qk.rearrange("(m p) -> p m", p=128))
        for k in range(ND):
            nc.sync.dma_start(out=wv_t[k], in_=w_v[128 * k:128 * (k + 1)])
        for k in range(NK3):
            nc.sync.dma_start(out=wo_t[k], in_=wo_T[128 * k:128 * (k + 1)])
        for i in range(n_strips):
            nc.sync.dma_start(out=mm_t[i], in_=maskm[i])
        for i in range(n_biases):
            nc.sync.dma_start(out=mb_t[i], in_=maskb[i])
        nc.vector.memset(ones_col, 1.0)
        for t in range(N_KT):
            nc.vector.memset(v_sb[t][:, :, 0:1], 1.0)
            nc.vector.memset(v_sb[t][:, :, 1:DK], 0.0)

        # ==== projection for seq block 0: transient 8-bank k-outer layout
        # (first matmul only waits for xs[0] + wqk[0]) ====
        with tc.tile_pool(name="p1ps0", bufs=8, space="PSUM") as p1ps0:
            pss = [p1ps0.tile([128, SB], f32, tag="p10", name=f"ps0{m}")
                   for m in range(NM)]
            for k in range(ND):
                for m in range(NM):
                    nc.tensor.matmul(
                        pss[m][:], wqk_t[k][:, 128 * m:128 * (m + 1)],
                        xs_cur[k][:], start=(k == 0), stop=(k == ND - 1))
            for m in range(NM):
                nc.vector.tensor_scalar_add(qkt[m][:, 0:SB], pss[m][:],
                                            bqk_t[:, m:m + 1])
            for tt in range(SB // 128):
                ps = p1ps0.tile([128, GD], f32, tag="p10", name="ps0_v")
                for k in range(ND):
                    nc.tensor.matmul(
                        ps[:], xs_cur[k][:, 128 * tt:128 * (tt + 1)], wv_t[k][:],
                        start=(k == 0), stop=(k == ND - 1))
                nc.vector.tensor_copy(
                    out=v_sb[tt][:, :, DK:2 * DK],
                    in_=ps[:].rearrange("p (h d) -> p h d", h=HPC))

        # ==== steady-state pools: 4 + 2 + 2 = 8 PSUM banks ====
        st_pool = ctx.enter_context(tc.tile_pool(name="st", bufs=2, space="PSUM"))
        ot_pool = ctx.enter_context(tc.tile_pool(name="ot", bufs=2, space="PSUM"))
        p1p = ctx.enter_context(tc.tile_pool(name="p1p", bufs=2, space="PSUM"))
        pt_pool = ctx.enter_context(tc.tile_pool(name="pt", bufs=6))
        rr_pool = ctx.enter_context(tc.tile_pool(name="rr", bufs=4))
        p3o = ctx.enter_context(tc.tile_pool(name="p3o", bufs=4))

        xs_next = [None]

        def p1_pieces(sb):
            """Projection for seq block sb as 12 single-PSUM-chunk pieces
            (m-outer, k-inner contraction into 1 bank at a time)."""
            xs = xs_next[0]

            def qk_piece(m=0, xs=xs, sb=sb):
                ps = p1p.tile([128, SB], f32, tag="p1", name="ps_qk")
                for k in range(ND):
                    nc.tensor.matmul(
                        ps[:], wqk_t[k][:, 128 * m:128 * (m + 1)], xs[k][:],
                        start=(k == 0), stop=(k == ND - 1))
                nc.vector.tensor_scalar_add(
                    qkt[m][:, SB * sb:SB * (sb + 1)], ps[:], bqk_t[:, m:m + 1])

            def v_piece(tt=0, xs=xs, sb=sb):
                t = sb * (SB // 128) + tt
                ps = p1p.tile([128, GD], f32, tag="p1", name="ps_v")
                for k in range(ND):
                    nc.tensor.matmul(
                        ps[:], xs[k][:, 128 * tt:128 * (tt + 1)], wv_t[k][:],
                        start=(k == 0), stop=(k == ND - 1))
                nc.vector.tensor_copy(
                    out=v_sb[t][:, :, DK:2 * DK],
                    in_=ps[:].rearrange("p (h d) -> p h d", h=HPC))

            for m in range(NM):
                yield (lambda m=m: qk_piece(m))
            for tt in range(SB // 128):
                yield (lambda tt=tt: v_piece(tt))

        def p3_pieces(qi, otq):
            """Output projection for q-tile qi as 8 single-chunk pieces."""
            def piece(m=0, qi=qi, otq=otq):
                ps = p1p.tile([128, QT], f32, tag="p1", name="ps_o")
                for k in range(NK3):
                    nc.tensor.matmul(
                        ps[:], wo_t[k][:, 128 * m:128 * (m + 1)], otq[k][:],
                        start=(k == 0), stop=(k == NK3 - 1))
                ob = p3o.tile([128, QT], f16, tag="ob", name="ob")
                nc.vector.tensor_copy(out=ob[:], in_=ps[:])
                nc.sync.dma_start(
                    out=outT[128 * m:128 * (m + 1), QT * qi:QT * (qi + 1)],
                    in_=ob[:])
            for m in range(D // 128):
                yield (lambda m=m: piece(m))

        # ==== attention q-tiles with woven projection pieces ====
        prev_otq = None
        do_norm = [lambda: None]
        for qi in range(N_QT):
            fill = []
            if qi + 1 < NSB:
                xs_next[0] = [xs_pool.tile([128, SB], f16, tag=f"x{k}",
                                           name=f"xs{k}") for k in range(ND)]
                for k in range(ND):
                    nc.sync.dma_start(
                        out=xs_next[0][k],
                        in_=xT[128 * k:128 * (k + 1),
                               SB * (qi + 1):SB * (qi + 2)])
                fill.extend(p1_pieces(qi + 1))
            if prev_otq is not None:
                fill.extend(p3_pieces(qi - 1, prev_otq))

            kts = plan[qi]
            n_blocks = max(len(kts) * (HPC // 2), 1)
            pace = len(fill) / n_blocks
            acc = 0.0
            fi = 0

            otq = [otq_pool.tile([128, QT], f16, tag=f"otq{m}", name=f"otq{m}")
                   for m in range(NK3)]
            for hp in range(HPC // 2):
                ot_ps = [ot_pool.tile([128, QT], f32, tag="ot", name="ot_ps")
                         for _ in range(2)]
                for ki, (kt, kind, a, bopt) in enumerate(kts):
                    if ki == 1:
                        do_norm[0]()
                    q0 = a if kind == 1 else 0
                    st = st_pool.tile([128, 2, QT], f32, tag="st", name="st")
                    for h in range(2):
                        lo, hi = 64 * h, 64 * h + 64
                        nc.tensor.matmul(
                            st[:, h, q0:QT],
                            qkt[4 + hp][lo:hi, KTL * kt:KTL * (kt + 1)],
                            qkt[hp][lo:hi, QT * qi + q0:QT * (qi + 1)],
                            start=True, stop=True, tile_position=(64 * h, 0))
                    # weave projection pieces while EXP runs on ScalarE
                    acc += pace
                    while acc >= 1.0 and fi < len(fill):
                        fill[fi]()
                        fi += 1
                        acc -= 1.0
                    if kind == 2:
                        for h in range(2):
                            nc.vector.tensor_add(st[:, h, :], st[:, h, :],
                                                 mb_t[a][:])
                    pt = pt_pool.tile([128, 2, QT], f16, tag="pt", name="pt")
                    nc.scalar.activation(out=pt[:, :, q0:QT], in_=st[:, :, q0:QT],
                                         func=Exp, scale=float(SCALE))
                    if kind == 1:
                        si, w = bopt
                        nc.vector.tensor_mul(
                            pt[:, :, q0:q0 + w], pt[:, :, q0:q0 + w],
                            mm_t[si][:, None, 0:w].broadcast_to([KTL, 2, w]))
                    for h in range(2):
                        nc.tensor.matmul(
                            ot_ps[h][:, q0:QT],
                            v_sb[kt][:, 2 * hp + h, :],
                            pt[:, h, q0:QT],
                            start=(ki == 0), stop=(ki == len(kts) - 1))
                # softmax normalization: 1/denom broadcast via K=1 matmul.
                # Deferred: emitted after the next head-pair's first block so
                # the PE has QK work while DVE computes the reciprocal.
                def norm(ot_ps=ot_ps, dst=otq[hp]):
                    r16s = []
                    for h in range(2):
                        r_row = rr_pool.tile([1, QT], f32, tag=f"rrow{h}",
                                             name="r_row")
                        nc.vector.reciprocal_approx_fast(out=r_row[:],
                                                         in_=ot_ps[h][0:1, :])
                        r16 = rr_pool.tile([1, QT], f16, tag=f"r16{h}",
                                           name="r16")
                        nc.vector.tensor_copy(out=r16[:], in_=r_row[:])
                        r16s.append(r16)
                    for h in range(2):
                        rb_ps = p1p.tile([DK, QT], f32, tag="p1", name="rb_ps")
                        nc.tensor.matmul(rb_ps[:], ones_col[:], r16s[h][:],
                                         start=True, stop=True)
                        rb_sb = rr_pool.tile([DK, QT], f32, tag=f"rbsb{h}",
                                             name="rb_sb")
                        nc.vector.tensor_copy(out=rb_sb[:], in_=rb_ps[:])
                        nc.vector.tensor_mul(dst[64 * h:64 * h + 64, :],
                                             ot_ps[h][DK:2 * DK, :], rb_sb[:])
                    do_norm[0] = (lambda: None)
                do_norm[0] = norm
            while fi < len(fill):
                fill[fi]()
                fi += 1
            prev_otq = otq
        # final q-tile's output projection
        do_norm[0]()
        for piece in p3_pieces(N_QT - 1, prev_otq):
            piece()
    nc.compile()
    return nc


def kernel(encodings_for_qkv, mask, w_qkv, b_qkv, w_o):
    global last_results
    from concourse.bass_utils import run_bass_kernel_spmd

    x = np.ascontiguousarray(np.asarray(encodings_for_qkv, dtype=np.float32))
    mask2d = np.asarray(mask).reshape(S, S).astype(bool)
    w_qkv = np.asarray(w_qkv, dtype=np.float32)
    b_qkv = np.asarray(b_qkv, dtype=np.float32)
    w_o = np.asarray(w_o, dtype=np.float32)

    plan, strips, biases = _classify_mask(mask2d)
    key = repr([[e[:3] + ((e[3][0], e[3][1]) if e[3] else None,) for e in row]
                for row in plan]) + repr(sorted(
                    (k, v) for k, v in os.environ.items() if k.startswith("KERNEL_")))
    if key not in _cache:
        _cache[key] = _build(plan, len(strips), len(biases))
    nc = _cache[key]

    maskm = (np.stack(strips) if strips
             else np.zeros((1, KTL, STRW), dtype=np.float32))
    maskb = (np.stack(biases) if biases
             else np.zeros((1, KTL, QT), dtype=np.float32))
    wT = np.ascontiguousarray(w_qkv.T)        # [D, 3D]
    woT_full = w_o.T                          # [D(in), D(out)]

    in_maps = []
    for c in range(8):
        b, g = divmod(c, 2)
        cols = slice(GD * g, GD * (g + 1))
        w_qk_g = np.ascontiguousarray(
            np.concatenate([wT[:, 0 * D:][:, cols], wT[:, 1 * D:][:, cols]], axis=1))
        b_qk_g = np.ascontiguousarray(
            np.concatenate([b_qkv[0 * D:1 * D][cols], b_qkv[1 * D:2 * D][cols]]))
        w_v_g = np.ascontiguousarray(wT[:, 2 * D:][:, cols])
        wo_T_g = np.ascontiguousarray(woT_full[cols, :])
        in_maps.append({
            "xT": np.ascontiguousarray(x[b].T).astype(np.float16),
            "w_qk": w_qk_g.astype(np.float16), "b_qk": b_qk_g,
            "w_v": w_v_g.astype(np.float16),
            "wo_T": wo_T_g.astype(np.float16),
            "maskm": maskm.astype(np.float16), "maskb": maskb,
        })

    trace = bool(int(os.environ.get("KERNEL_PROFILE", "0")))
    res = run_bass_kernel_spmd(nc, in_maps, core_ids=list(range(8)),
                               trace=trace,
                               trace_cores=list(range(8)) if trace else None)
    last_results = res

    out = np.empty((B, S, D), dtype=np.float32)
    for b in range(B):
        acc = (res.results[2 * b]["outT"].astype(np.float32)
               + res.results[2 * b + 1]["outT"].astype(np.float32))
        out[b] = acc.T
    # V-bias epilogue: softmax rows sum to 1, so the V bias contributes a
    # constant (b_v @ w_o.T) to every sequence position.
    out += (b_qkv[2 * D:] @ woT_full).reshape(1, 1, D)
    return out



# revision 16
# speedup vs baseline: 1.1244x; 1.0916x over previous
"""Multi-head attention (B=4, S=2048, D=1024, H=16) on 8 TRN2 NeuronCores.

Sharding: core c -> (batch b = c//2, head-group g = c%2 of 8 heads).
Data parallel over batch, tensor parallel over heads; each core computes
its group's QKV projection slices, causal attention for its 8 heads, and
the partial output projection. Host sums the two per-batch partials
(the tensor-parallel unshard) and adds the V-bias epilogue.

On-device layout is "features on partitions": x, Q, K arrive/stay
transposed [feat, seq]; attention scores are computed directly in
transposed form S.T[k, q] so the exp'd probabilities feed the PV matmul
without any on-chip transpose. The softmax denominator rides the PV
matmul as an appended ones-column of V; normalization is a fast
reciprocal + K=1 broadcast matmul + DVE multiply.

Scheduling: TensorE executes in program order, so the QKV projection for
seq block sb+1 and the output projection for q-tile qi-1 are emitted as
small 2-PSUM-bank pieces woven between attention blocks of q-tile qi.
That keeps the PE busy while the (otherwise pacing) softmax EXP runs on
ScalarE, instead of running projection and attention as serial phases.
The first seq block's projection uses a transient 8-bank PSUM layout so
the first matmul only waits for one x/w chunk pair.
"""

import os
import numpy as np

B, S, D, H = 4, 2048, 1024, 16
DK = D // H          # 64
HPC = H // 2         # heads per core = 8
GD = HPC * DK        # group feature width = 512
QT = 512             # q-tile width (free dim of S.T chunks)
KTL = 128            # k-tile length (partition dim of S.T chunks)
N_QT = S // QT       # 4
N_KT = S // KTL      # 16
SB = 512             # projection seq block
STRW = 256           # padded width of multiplicative mask strips
NEG = np.float32(-1e9)
SCALE = 1.0 / np.sqrt(np.float32(DK))

_cache = {}
last_results = None


def _classify_mask(mask2d):
    """Classify each (q-tile, k-tile) block of the [S,S] bool mask.

    Returns (plan, strips, biases):
      plan[qi] = list over valid kt of (kt, kind, a, b):
        kind 0 = clean (no masking)
        kind 1 = staircase: a = q0 (suffix start), b = (strip_idx, strip_w)
        kind 2 = general:   a = bias_idx
      strips: list of [KTL, STRW] f32 0/1 multiplicative masks (padded)
      biases: list of [KTL, QT] f32 additive -1e9/0 masks
    Blocks are in S.T (k, q) layout.
    """
    kl = np.arange(KTL)[:, None]
    ql = np.arange(QT)[None, :]
    plan = []
    strips, strip_keys = [], {}
    biases, bias_keys = [], {}
    for qi in range(N_QT):
        row = []
        for kt in range(N_KT):
            blk = mask2d[qi * QT:(qi + 1) * QT, kt * KTL:(kt + 1) * KTL].T
            if blk.all():
                continue
            if not blk.any():
                row.append((kt, 0, 0, None))
                continue
            dj = kt * KTL - qi * QT
            stair = (0 <= dj <= QT - KTL and np.array_equal(blk, kl + dj > ql)
                     and not os.environ.get("KERNEL_NO_STAIR"))
            q0 = min(dj, QT - STRW) if stair else 0
            if stair and (q0 == 0 or row):
                w = min(dj + KTL, QT) - q0
                pat = (~blk[:, q0:q0 + w]).astype(np.float32)
                key = (w, pat.tobytes())
                if key not in strip_keys:
                    strip_keys[key] = len(strips)
                    p = np.zeros((KTL, STRW), np.float32)
                    p[:, :w] = pat
                    strips.append(p)
                row.append((kt, 1, q0, (strip_keys[key], w)))
            else:
                bias = np.where(blk, NEG, np.float32(0.0))
                key = bias.tobytes()
                if key not in bias_keys:
                    bias_keys[key] = len(biases)
                    biases.append(bias)
                row.append((kt, 2, bias_keys[key], None))
        if not row:
            # fully-masked q-row: include everything with full bias so the
            # softmax matches the reference's uniform distribution.
            bias = np.full((KTL, QT), NEG, np.float32)
            key = bias.tobytes()
            if key not in bias_keys:
                bias_keys[key] = len(biases)
                biases.append(bias)
            row = [(kt, 2, bias_keys[key], None) for kt in range(N_KT)]
        plan.append(row)
    return plan, strips, biases


def _build(plan, n_strips, n_biases):
    import concourse.bacc as bacc
    import concourse.tile as tile
    import concourse.mybir as mybir
    from contextlib import ExitStack

    f32 = mybir.dt.float32
    f16 = mybir.dt.float16
    Exp = mybir.ActivationFunctionType.Exp

    nc = bacc.Bacc(trn_type="TRN2", target_bir_lowering=False, debug=False)
    xT = nc.dram_tensor("xT", [D, S], f16, kind="ExternalInput").ap()
    w_qk = nc.dram_tensor("w_qk", [D, 2 * GD], f16, kind="ExternalInput").ap()
    b_qk = nc.dram_tensor("b_qk", [2 * GD], f32, kind="ExternalInput").ap()
    w_v = nc.dram_tensor("w_v", [D, GD], f16, kind="ExternalInput").ap()
    wo_T = nc.dram_tensor("wo_T", [GD, D], f16, kind="ExternalInput").ap()
    maskm = nc.dram_tensor("maskm", [max(n_strips, 1), KTL, STRW], f16,
                           kind="ExternalInput").ap()
    maskb = nc.dram_tensor("maskb", [max(n_biases, 1), KTL, QT], f32,
                           kind="ExternalInput").ap()
    outT = nc.dram_tensor("outT", [D, S], f16, kind="ExternalOutput").ap()

    ND = D // 128       # 8 contraction chunks
    NM = 2 * GD // 128  # 8 QK feature chunks (0-3 = Q.T, 4-7 = K.T)
    NK3 = GD // 128     # 4 output-projection contraction chunks
    NSB = S // SB       # 4

    with tile.TileContext(nc) as tc, ExitStack() as ctx:
        singles = ctx.enter_context(tc.tile_pool(name="singles", bufs=1))
        qkt_pool = ctx.enter_context(tc.tile_pool(name="qkt", bufs=1))
        v_pool = ctx.enter_context(tc.tile_pool(name="vp", bufs=1))
        otq_pool = ctx.enter_context(tc.tile_pool(name="otq", bufs=2))
        xs_pool = ctx.enter_context(tc.tile_pool(name="p1x", bufs=2))

        qkt = [qkt_pool.tile([128, S], f16, tag=f"qkt{m}", name=f"qkt{m}")
               for m in range(NM)]
        ones_col = singles.tile([1, DK], f16)
        v_sb = [v_pool.tile([128, HPC, 128], f16, tag=f"v{t}", name=f"v{t}")
                for t in range(N_KT)]
        bqk_t = singles.tile([128, NM], f32)
        wqk_t = [singles.tile([128, 2 * GD], f16, tag=f"wqk{k}", name=f"wqk{k}")
                 for k in range(ND)]
        wv_t = [singles.tile([128, GD], f16, tag=f"wv{k}", name=f"wv{k}")
                for k in range(ND)]
        wo_t = [singles.tile([128, D], f16, tag=f"wo{k}", name=f"wo{k}")
                for k in range(NK3)]
        mm_t = [singles.tile([KTL, STRW], f16, tag=f"mm{i}", name=f"mm{i}")
                for i in range(n_strips)]
        mb_t = [singles.tile([KTL, QT], f32, tag=f"mb{i}", name=f"mb{i}")
                for i in range(n_biases)]

        # Load order matters: the first seq block's projection contracts
        # k-outer, so (xs[k], wqk[k]) pairs in k order come first; wv/wo/
        # masks follow and finish while the QK matmuls run.
        xs_cur = [xs_pool.tile([128, SB], f16, tag=f"x{k}", name=f"xs{k}")
                  for k in range(ND)]
        for k in range(ND):
            nc.sync.dma_start(out=xs_cur[k], in_=xT[128 * k:128 * (k + 1), 0:SB])
            nc.sync.dma_start(out=wqk_t[k], in_=w_qk[128 * k:128 * (k + 1)])
        nc.sync.dma_start(out=bqk_t, in_=b_qk.rearrange("(m p) -> p m", p=128))
        for k in range(ND):
            nc.sync.dma_start(out=wv_t[k], in_=w_v[128 * k:128 * (k + 1)])
        for k in range(NK3):
            nc.sync.dma_start(out=wo_t[k], in_=wo_T[128 * k:128 * (k + 1)])
        for i in range(n_strips):
            nc.sync.dma_start(out=mm_t[i], in_=maskm[i])
        for i in range(n_biases):
            nc.sync.dma_start(out=mb_t[i], in_=maskb[i])
        nc.vector.memset(ones_col, 1.0)
        for t in range(N_KT):
            nc.vector.memset(v_sb[t][:, :, 0:1], 1.0)
            nc.vector.memset(v_sb[t][:, :, 1:DK], 0.0)

        # ==== projection for seq block 0: transient 8-bank k-outer layout
        # (first matmul only waits for xs[0] + wqk[0]) ====
        with tc.tile_pool(name="p1ps0", bufs=8, space="PSUM") as p1ps0:
            pss = [p1ps0.tile([128, SB], f32, tag="p10", name=f"ps0{m}")
                   for m in range(NM)]
            for k in range(ND):
                for m in range(NM):
                    nc.tensor.matmul(
                        pss[m][:], wqk_t[k][:, 128 * m:128 * (m + 1)],
                        xs_cur[k][:], start=(k == 0), stop=(k == ND - 1))
            for m in range(NM):
                nc.vector.tensor_scalar_add(qkt[m][:, 0:SB], pss[m][:],
                                            bqk_t[:, m:m + 1])
            for tt in range(SB // 128):
                ps = p1ps0.tile([128, GD], f32, tag="p10", name="ps0_v")
                for k in range(ND):
                    nc.tensor.matmul(
                        ps[:], xs_cur[k][:, 128 * tt:128 * (tt + 1)], wv_t[k][:],
                        start=(k == 0), stop=(k == ND - 1))
                nc.vector.tensor_copy(
                    out=v_sb[tt][:, :, DK:2 * DK],
                    in_=ps[:].rearrange("p (h d) -> p h d", h=HPC))

        # ==== steady-state pools: 4 + 2 + 2 = 8 PSUM banks ====
        st_pool = ctx.enter_context(tc.tile_pool(name="st", bufs=2, space="PSUM"))
        ot_pool = ctx.enter_context(tc.tile_pool(name="ot", bufs=2, space="PSUM"))
        p1p = ctx.enter_context(tc.tile_pool(name="p1p", bufs=2, space="PSUM"))
        pt_pool = ctx.enter_context(tc.tile_pool(name="pt", bufs=6))
        rr_pool = ctx.enter_context(tc.tile_pool(name="rr", bufs=4))
        p3o = ctx.enter_context(tc.tile_pool(name="p3o", bufs=4))

        xs_next = [None]

        def p1_pieces(sb):
            """Projection for seq block sb as 12 single-PSUM-chunk pieces
            (m-outer, k-inner contraction into 1 bank at a time)."""
            xs = xs_next[0]

            def qk_piece(m=0, xs=xs, sb=sb):
                ps = p1p.tile([128, SB], f32, tag="p1", name="ps_qk")
                for k in range(ND):
                    nc.tensor.matmul(
                        ps[:], wqk_t[k][:, 128 * m:128 * (m + 1)], xs[k][:],
                        start=(k == 0), stop=(k == ND - 1))
                nc.vector.tensor_scalar_add(
                    qkt[m][:, SB * sb:SB * (sb + 1)], ps[:], bqk_t[:, m:m + 1])

            def v_piece(tt=0, xs=xs, sb=sb):
                t = sb * (SB // 128) + tt
                ps = p1p.tile([128, GD], f32, tag="p1", name="ps_v")
                for k in range(ND):
                    nc.tensor.matmul(
                        ps[:], xs[k][:, 128 * tt:128 * (tt + 1)], wv_t[k][:],
                        start=(k == 0), stop=(k == ND - 1))
                nc.vector.tensor_copy(
                    out=v_sb[t][:, :, DK:2 * DK],
                    in_=ps[:].rearrange("p (h d) -> p h d", h=HPC))

            for m in range(NM):
                yield (lambda m=m: qk_piece(m))
            for tt in range(SB // 128):
                yield (lambda tt=tt: v_piece(tt))

        def p3_pieces(qi, otq):
            """Output projection for q-tile qi as 8 single-chunk pieces."""
            def piece(m=0, qi=qi, otq=otq):
                ps = p1p.tile([128, QT], f32, tag="p1", name="ps_o")
                for k in range(NK3):
                    nc.tensor.matmul(
                        ps[:], wo_t[k][:, 128 * m:128 * (m + 1)], otq[k][:],
                        start=(k == 0), stop=(k == NK3 - 1))
                ob = p3o.tile([128, QT], f16, tag="ob", name="ob")
                nc.vector.tensor_copy(out=ob[:], in_=ps[:])
                nc.sync.dma_start(
                    out=outT[128 * m:128 * (m + 1), QT * qi:QT * (qi + 1)],
                    in_=ob[:])
            for m in range(D // 128):
                yield (lambda m=m: piece(m))

        # ==== attention q-tiles with woven projection pieces ====
        prev_otq = None
        for qi in range(N_QT):
            fill = []
            if qi + 1 < NSB:
                xs_next[0] = [xs_pool.tile([128, SB], f16, tag=f"x{k}",
                                           name=f"xs{k}") for k in range(ND)]
                for k in range(ND):
                    nc.sync.dma_start(
                        out=xs_next[0][k],
                        in_=xT[128 * k:128 * (k + 1),
                               SB * (qi + 1):SB * (qi + 2)])
                fill.extend(p1_pieces(qi + 1))
            if prev_otq is not None:
                fill.extend(p3_pieces(qi - 1, prev_otq))

            kts = plan[qi]
            n_blocks = max(len(kts) * (HPC // 2), 1)
            pace = len(fill) / n_blocks
            acc = 0.0
            fi = 0

            otq = [otq_pool.tile([128, QT], f16, tag=f"otq{m}", name=f"otq{m}")
                   for m in range(NK3)]
            for hp in range(HPC // 2):
                ot_ps = [ot_pool.tile([128, QT], f32, tag="ot", name="ot_ps")
                         for _ in range(2)]
                for ki, (kt, kind, a, bopt) in enumerate(kts):
                    q0 = a if kind == 1 else 0
                    st = st_pool.tile([128, 2, QT], f32, tag="st", name="st")
                    for h in range(2):
                        lo, hi = 64 * h, 64 * h + 64
                        nc.tensor.matmul(
                            st[:, h, q0:QT],
                            qkt[4 + hp][lo:hi, KTL * kt:KTL * (kt + 1)],
                            qkt[hp][lo:hi, QT * qi + q0:QT * (qi + 1)],
                            start=True, stop=True, tile_position=(64 * h, 0))
                    # weave projection pieces while EXP runs on ScalarE
                    acc += pace
                    while acc >= 1.0 and fi < len(fill):
                        fill[fi]()
                        fi += 1
                        acc -= 1.0
                    if kind == 2:
                        for h in range(2):
                            nc.vector.tensor_add(st[:, h, :], st[:, h, :],
                                                 mb_t[a][:])
                    pt = pt_pool.tile([128, 2, QT], f16, tag="pt", name="pt")
                    nc.scalar.activation(out=pt[:, :, q0:QT], in_=st[:, :, q0:QT],
                                         func=Exp, scale=float(SCALE))
                    if kind == 1:
                        si, w = bopt
                        nc.vector.tensor_mul(
                            pt[:, :, q0:q0 + w], pt[:, :, q0:q0 + w],
                            mm_t[si][:, None, 0:w].broadcast_to([KTL, 2, w]))
                    for h in range(2):
                        nc.tensor.matmul(
                            ot_ps[h][:, q0:QT],
                            v_sb[kt][:, 2 * hp + h, :],
                            pt[:, h, q0:QT],
                            start=(ki == 0), stop=(ki == len(kts) - 1))
                # softmax normalization. All PSUM reads come first (2
                # reciprocals + 2 stage copies) so the ot banks free fast;
                # the 1/denom row broadcast runs on the idle GpSimd engine,
                # keeping the PE instruction stream free of norm work.
                r_rows, osbs = [], []
                for h in range(2):
                    r_row = rr_pool.tile([1, QT], f32, tag=f"rrow{h}",
                                         name="r_row")
                    nc.vector.reciprocal_approx_fast(out=r_row[:],
                                                     in_=ot_ps[h][0:1, :])
                    r_rows.append(r_row)
                    osb = rr_pool.tile([DK, QT], f16, tag=f"osb{h}",
                                       name="osb")
                    nc.vector.tensor_copy(out=osb[:], in_=ot_ps[h][DK:2 * DK, :])
                    osbs.append(osb)
                for h in range(2):
                    r16 = rr_pool.tile([1, QT], f16, tag=f"r16{h}", name="r16")
                    nc.vector.tensor_copy(out=r16[:], in_=r_rows[h][:])
                    rb16 = rr_pool.tile([DK, QT], f16, tag=f"rb{h}", name="rb16")
                    nc.gpsimd.partition_broadcast(rb16[:], r16[:], channels=DK)
                    nc.vector.tensor_mul(otq[hp][64 * h:64 * h + 64, :],
                                         osbs[h][:], rb16[:])
            while fi < len(fill):
                fill[fi]()
                fi += 1
            prev_otq = otq
        # final q-tile's output projection
        for piece in p3_pieces(N_QT - 1, prev_otq):
            piece()
    nc.compile()
    return nc


def kernel(encodings_for_qkv, mask, w_qkv, b_qkv, w_o):
    global last_results
    from concourse.bass_utils import run_bass_kernel_spmd

    x = np.ascontiguousarray(np.asarray(encodings_for_qkv, dtype=np.float32))
    mask2d = np.asarray(mask).reshape(S, S).astype(bool)
    w_qkv = np.asarray(w_qkv, dtype=np.float32)
    b_qkv = np.asarray(b_qkv, dtype=np.float32)
    w_o = np.asarray(w_o, dtype=np.float32)

    plan, strips, biases = _classify_mask(mask2d)
    key = repr([[e[:3] + ((e[3][0], e[3][1]) if e[3] else None,) for e in row]
                for row in plan]) + repr(sorted(
                    (k, v) for k, v in os.environ.items() if k.startswith("KERNEL_")))
    if key not in _cache:
        _cache[key] = _build(plan, len(strips), len(biases))
    nc = _cache[key]

    maskm = (np.stack(strips) if strips
             else np.zeros((1, KTL, STRW), dtype=np.float32))
    maskb = (np.stack(biases) if biases
             else np.zeros((1, KTL, QT), dtype=np.float32))
    wT = np.ascontiguousarray(w_qkv.T)        # [D, 3D]
    woT_full = w_o.T                          # [D(in), D(out)]

    in_maps = []
    for c in range(8):
        b, g = divmod(c, 2)
        cols = slice(GD * g, GD * (g + 1))
        w_qk_g = np.ascontiguousarray(
            np.concatenate([wT[:, 0 * D:][:, cols], wT[:, 1 * D:][:, cols]], axis=1))
        b_qk_g = np.ascontiguousarray(
            np.concatenate([b_qkv[0 * D:1 * D][cols], b_qkv[1 * D:2 * D][cols]]))
        w_v_g = np.ascontiguousarray(wT[:, 2 * D:][:, cols])
        wo_T_g = np.ascontiguousarray(woT_full[cols, :])
        in_maps.append({
            "xT": np.ascontiguousarray(x[b].T).astype(np.float16),
            "w_qk": w_qk_g.astype(np.float16), "b_qk": b_qk_g,
            "w_v": w_v_g.astype(np.float16),
            "wo_T": wo_T_g.astype(np.float16),
            "maskm": maskm.astype(np.float16), "maskb": maskb,
        })

    trace = bool(int(os.environ.get("KERNEL_PROFILE", "0")))
    res = run_bass_kernel_spmd(nc, in_maps, core_ids=list(range(8)),
                               trace=trace,
                               trace_cores=list(range(8)) if trace else None)
    last_results = res

    out = np.empty((B, S, D), dtype=np.float32)
    for b in range(B):
        acc = (res.results[2 * b]["outT"].astype(np.float32)
               + res.results[2 * b + 1]["outT"].astype(np.float32))
        out[b] = acc.T
    # V-bias epilogue: softmax rows sum to 1, so the V bias contributes a
    # constant (b_v @ w_o.T) to every sequence position.
    out += (b_qkv[2 * D:] @ woT_full).reshape(1, 1, D)
    return out



# revision 17
# speedup vs baseline: 1.1475x; 1.0206x over previous
"""Multi-head attention (B=4, S=2048, D=1024, H=16) on 8 TRN2 NeuronCores.

Sharding: core c -> (batch b = c//2, head-group g = c%2 of 8 heads).
Data parallel over batch, tensor parallel over heads; each core computes
its group's QKV projection slices, causal attention for its 8 heads, and
the partial output projection. Host sums the two per-batch partials
(the tensor-parallel unshard) and adds the V-bias epilogue.

On-device layout is "features on partitions": x, Q, K arrive/stay
transposed [feat, seq]; attention scores are computed directly in
transposed form S.T[k, q] so the exp'd probabilities feed the PV matmul
without any on-chip transpose. The softmax denominator rides the PV
matmul as an appended ones-column of V; normalization is a fast
reciprocal + K=1 broadcast matmul + DVE multiply.

Scheduling: TensorE executes in program order, so the QKV projection for
seq block sb+1 and the output projection for q-tile qi-1 are emitted as
small 2-PSUM-bank pieces woven between attention blocks of q-tile qi.
That keeps the PE busy while the (otherwise pacing) softmax EXP runs on
ScalarE, instead of running projection and attention as serial phases.
The first seq block's projection uses a transient 8-bank PSUM layout so
the first matmul only waits for one x/w chunk pair.
"""

import os
import numpy as np

B, S, D, H = 4, 2048, 1024, 16
DK = D // H          # 64
HPC = H // 2         # heads per core = 8
GD = HPC * DK        # group feature width = 512
QT = 512             # q-tile width (free dim of S.T chunks)
KTL = 128            # k-tile length (partition dim of S.T chunks)
N_QT = S // QT       # 4
N_KT = S // KTL      # 16
SB = 512             # projection seq block
STRW = 256           # padded width of multiplicative mask strips
NEG = np.float32(-1e9)
SCALE = 1.0 / np.sqrt(np.float32(DK))

_cache = {}
last_results = None


def _classify_mask(mask2d):
    """Classify each (q-tile, k-tile) block of the [S,S] bool mask.

    Returns (plan, strips, biases):
      plan[qi] = list over valid kt of (kt, kind, a, b):
        kind 0 = clean (no masking)
        kind 1 = staircase: a = q0 (suffix start), b = (strip_idx, strip_w)
        kind 2 = general:   a = bias_idx
      strips: list of [KTL, STRW] f32 0/1 multiplicative masks (padded)
      biases: list of [KTL, QT] f32 additive -1e9/0 masks
    Blocks are in S.T (k, q) layout.
    """
    kl = np.arange(KTL)[:, None]
    ql = np.arange(QT)[None, :]
    plan = []
    strips, strip_keys = [], {}
    biases, bias_keys = [], {}
    for qi in range(N_QT):
        row = []
        for kt in range(N_KT):
            blk = mask2d[qi * QT:(qi + 1) * QT, kt * KTL:(kt + 1) * KTL].T
            if blk.all():
                continue
            if not blk.any():
                row.append((kt, 0, 0, None))
                continue
            dj = kt * KTL - qi * QT
            stair = (0 <= dj <= QT - KTL and np.array_equal(blk, kl + dj > ql)
                     and not os.environ.get("KERNEL_NO_STAIR"))
            q0 = min(dj, QT - STRW) if stair else 0
            if stair and (q0 == 0 or row):
                w = min(dj + KTL, QT) - q0
                pat = (~blk[:, q0:q0 + w]).astype(np.float32)
                key = (w, pat.tobytes())
                if key not in strip_keys:
                    strip_keys[key] = len(strips)
                    p = np.zeros((KTL, STRW), np.float32)
                    p[:, :w] = pat
                    strips.append(p)
                row.append((kt, 1, q0, (strip_keys[key], w)))
            else:
                bias = np.where(blk, NEG, np.float32(0.0))
                key = bias.tobytes()
                if key not in bias_keys:
                    bias_keys[key] = len(biases)
                    biases.append(bias)
                row.append((kt, 2, bias_keys[key], None))
        if not row:
            # fully-masked q-row: include everything with full bias so the
            # softmax matches the reference's uniform distribution.
            bias = np.full((KTL, QT), NEG, np.float32)
            key = bias.tobytes()
            if key not in bias_keys:
                bias_keys[key] = len(biases)
                biases.append(bias)
            row = [(kt, 2, bias_keys[key], None) for kt in range(N_KT)]
        plan.append(row)
    return plan, strips, biases


def _build(plan, n_strips, n_biases):
    import concourse.bacc as bacc
    import concourse.tile as tile
    import concourse.mybir as mybir
    from contextlib import ExitStack

    f32 = mybir.dt.float32
    f16 = mybir.dt.float16
    Exp = mybir.ActivationFunctionType.Exp

    nc = bacc.Bacc(trn_type="TRN2", target_bir_lowering=False, debug=False)
    xT = nc.dram_tensor("xT", [D, S], f16, kind="ExternalInput").ap()
    w_qk = nc.dram_tensor("w_qk", [D, 2 * GD], f16, kind="ExternalInput").ap()
    b_qk = nc.dram_tensor("b_qk", [2 * GD], f32, kind="ExternalInput").ap()
    w_v = nc.dram_tensor("w_v", [D, GD], f16, kind="ExternalInput").ap()
    wo_T = nc.dram_tensor("wo_T", [GD, D], f16, kind="ExternalInput").ap()
    maskm = nc.dram_tensor("maskm", [max(n_strips, 1), KTL, STRW], f16,
                           kind="ExternalInput").ap()
    maskb = nc.dram_tensor("maskb", [max(n_biases, 1), KTL, QT], f32,
                           kind="ExternalInput").ap()
    outT = nc.dram_tensor("outT", [D, S], f16, kind="ExternalOutput").ap()

    ND = D // 128       # 8 contraction chunks
    NM = 2 * GD // 128  # 8 QK feature chunks (0-3 = Q.T, 4-7 = K.T)
    NK3 = GD // 128     # 4 output-projection contraction chunks
    NSB = S // SB       # 4

    with tile.TileContext(nc) as tc, ExitStack() as ctx:
        singles = ctx.enter_context(tc.tile_pool(name="singles", bufs=1))
        qkt_pool = ctx.enter_context(tc.tile_pool(name="qkt", bufs=1))
        v_pool = ctx.enter_context(tc.tile_pool(name="vp", bufs=1))
        otq_pool = ctx.enter_context(tc.tile_pool(name="otq", bufs=3))
        xs_pool = ctx.enter_context(tc.tile_pool(name="p1x", bufs=2))

        qkt = [qkt_pool.tile([128, S], f16, tag=f"qkt{m}", name=f"qkt{m}")
               for m in range(NM)]
        ones_col = singles.tile([1, DK], f16)
        v_sb = [v_pool.tile([128, HPC, 128], f16, tag=f"v{t}", name=f"v{t}")
                for t in range(N_KT)]
        bqk_t = singles.tile([128, NM], f32)
        wqk_t = [singles.tile([128, 2 * GD], f16, tag=f"wqk{k}", name=f"wqk{k}")
                 for k in range(ND)]
        wv_t = [singles.tile([128, GD], f16, tag=f"wv{k}", name=f"wv{k}")
                for k in range(ND)]
        wo_t = [singles.tile([128, D], f16, tag=f"wo{k}", name=f"wo{k}")
                for k in range(NK3)]
        mm_t = [singles.tile([KTL, STRW], f16, tag=f"mm{i}", name=f"mm{i}")
                for i in range(n_strips)]
        mb_t = [singles.tile([KTL, QT], f32, tag=f"mb{i}", name=f"mb{i}")
                for i in range(n_biases)]

        # Load order matters: the first seq block's projection contracts
        # k-outer, so (xs[k], wqk[k]) pairs in k order come first; wv/wo/
        # masks follow and finish while the QK matmuls run.
        xs_cur = [xs_pool.tile([128, SB], f16, tag=f"x{k}", name=f"xs{k}")
                  for k in range(ND)]
        for k in range(ND):
            nc.sync.dma_start(out=xs_cur[k], in_=xT[128 * k:128 * (k + 1), 0:SB])
            nc.sync.dma_start(out=wqk_t[k], in_=w_qk[128 * k:128 * (k + 1)])
        nc.sync.dma_start(out=bqk_t, in_=b_qk.rearrange("(m p) -> p m", p=128))
        for k in range(ND):
            nc.sync.dma_start(out=wv_t[k], in_=w_v[128 * k:128 * (k + 1)])
        for k in range(NK3):
            nc.sync.dma_start(out=wo_t[k], in_=wo_T[128 * k:128 * (k + 1)])
        for i in range(n_strips):
            nc.sync.dma_start(out=mm_t[i], in_=maskm[i])
        for i in range(n_biases):
            nc.sync.dma_start(out=mb_t[i], in_=maskb[i])
        nc.vector.memset(ones_col, 1.0)
        for t in range(N_KT):
            nc.vector.memset(v_sb[t][:, :, 0:1], 1.0)
            nc.vector.memset(v_sb[t][:, :, 1:DK], 0.0)

        # ==== projection for seq block 0: transient 8-bank k-outer layout
        # (first matmul only waits for xs[0] + wqk[0]) ====
        with tc.tile_pool(name="p1ps0", bufs=8, space="PSUM") as p1ps0:
            pss = [p1ps0.tile([128, SB], f32, tag="p10", name=f"ps0{m}")
                   for m in range(NM)]
            for k in range(ND):
                for m in range(NM):
                    nc.tensor.matmul(
                        pss[m][:], wqk_t[k][:, 128 * m:128 * (m + 1)],
                        xs_cur[k][:], start=(k == 0), stop=(k == ND - 1))
            for m in range(NM):
                nc.vector.tensor_scalar_add(qkt[m][:, 0:SB], pss[m][:],
                                            bqk_t[:, m:m + 1])
            for tt in range(SB // 128):
                ps = p1ps0.tile([128, GD], f32, tag="p10", name="ps0_v")
                for k in range(ND):
                    nc.tensor.matmul(
                        ps[:], xs_cur[k][:, 128 * tt:128 * (tt + 1)], wv_t[k][:],
                        start=(k == 0), stop=(k == ND - 1))
                nc.vector.tensor_copy(
                    out=v_sb[tt][:, :, DK:2 * DK],
                    in_=ps[:].rearrange("p (h d) -> p h d", h=HPC))

        # ==== steady-state pools: 4 + 2 + 2 = 8 PSUM banks ====
        st_pool = ctx.enter_context(tc.tile_pool(name="st", bufs=2, space="PSUM"))
        ot_pool = ctx.enter_context(tc.tile_pool(name="ot", bufs=2, space="PSUM"))
        p1p = ctx.enter_context(tc.tile_pool(name="p1p", bufs=2, space="PSUM"))
        pt_pool = ctx.enter_context(tc.tile_pool(name="pt", bufs=6))
        rr_pool = ctx.enter_context(tc.tile_pool(name="rr", bufs=4))
        p3o = ctx.enter_context(tc.tile_pool(name="p3o", bufs=4))

        xs_next = [None]

        def p1_pieces(sb):
            """Projection for seq block sb as 12 single-PSUM-chunk pieces
            (m-outer, k-inner contraction into 1 bank at a time)."""
            xs = xs_next[0]

            def qk_piece(m=0, xs=xs, sb=sb):
                ps = p1p.tile([128, SB], f32, tag="p1", name="ps_qk")
                for k in range(ND):
                    nc.tensor.matmul(
                        ps[:], wqk_t[k][:, 128 * m:128 * (m + 1)], xs[k][:],
                        start=(k == 0), stop=(k == ND - 1))
                nc.vector.tensor_scalar_add(
                    qkt[m][:, SB * sb:SB * (sb + 1)], ps[:], bqk_t[:, m:m + 1])

            def v_piece(tt=0, xs=xs, sb=sb):
                t = sb * (SB // 128) + tt
                ps = p1p.tile([128, GD], f32, tag="p1", name="ps_v")
                for k in range(ND):
                    nc.tensor.matmul(
                        ps[:], xs[k][:, 128 * tt:128 * (tt + 1)], wv_t[k][:],
                        start=(k == 0), stop=(k == ND - 1))
                nc.vector.tensor_copy(
                    out=v_sb[t][:, :, DK:2 * DK],
                    in_=ps[:].rearrange("p (h d) -> p h d", h=HPC))

            for m in range(NM):
                yield (lambda m=m: qk_piece(m))
            for tt in range(SB // 128):
                yield (lambda tt=tt: v_piece(tt))

        def p3_pieces(qi, otq):
            """Output projection for q-tile qi as 8 single-chunk pieces."""
            def piece(m=0, qi=qi, otq=otq):
                ps = p1p.tile([128, QT], f32, tag="p1", name="ps_o")
                for k in range(NK3):
                    nc.tensor.matmul(
                        ps[:], wo_t[k][:, 128 * m:128 * (m + 1)], otq[k][:],
                        start=(k == 0), stop=(k == NK3 - 1))
                ob = p3o.tile([128, QT], f16, tag="ob", name="ob")
                nc.vector.tensor_copy(out=ob[:], in_=ps[:])
                nc.sync.dma_start(
                    out=outT[128 * m:128 * (m + 1), QT * qi:QT * (qi + 1)],
                    in_=ob[:])
            for m in range(D // 128):
                yield (lambda m=m: piece(m))

        # ==== attention q-tiles with woven projection pieces ====
        prev_otq = None
        otq_hist = {}
        for qi in range(N_QT):
            fill = []
            if qi + 1 < NSB:
                xs_next[0] = [xs_pool.tile([128, SB], f16, tag=f"x{k}",
                                           name=f"xs{k}") for k in range(ND)]
                for k in range(ND):
                    nc.sync.dma_start(
                        out=xs_next[0][k],
                        in_=xT[128 * k:128 * (k + 1),
                               SB * (qi + 1):SB * (qi + 2)])
                fill.extend(p1_pieces(qi + 1))
            # out-projections are deferred toward the late q-tiles where
            # attention rows are long and weave fill is otherwise scarce:
            # qi2 runs p3(qi0); qi3 runs p3(qi1) and p3(qi2).
            if qi == 2:
                fill.extend(p3_pieces(0, otq_hist[0]))
            elif qi == 3:
                fill.extend(p3_pieces(1, otq_hist[1]))
                fill.extend(p3_pieces(2, otq_hist[2]))

            kts = plan[qi]
            n_blocks = max(len(kts) * (HPC // 2), 1)
            pace = len(fill) / n_blocks
            acc = 0.0
            fi = 0

            otq = [otq_pool.tile([128, QT], f16, tag=f"otq{m}", name=f"otq{m}")
                   for m in range(NK3)]
            for hp in range(HPC // 2):
                ot_ps = [ot_pool.tile([128, QT], f32, tag="ot", name="ot_ps")
                         for _ in range(2)]
                for ki, (kt, kind, a, bopt) in enumerate(kts):
                    q0 = a if kind == 1 else 0
                    st = st_pool.tile([128, 2, QT], f32, tag="st", name="st")
                    for h in range(2):
                        lo, hi = 64 * h, 64 * h + 64
                        nc.tensor.matmul(
                            st[:, h, q0:QT],
                            qkt[4 + hp][lo:hi, KTL * kt:KTL * (kt + 1)],
                            qkt[hp][lo:hi, QT * qi + q0:QT * (qi + 1)],
                            start=True, stop=True, tile_position=(64 * h, 0))
                    # weave projection pieces while EXP runs on ScalarE
                    acc += pace
                    while acc >= 1.0 and fi < len(fill):
                        fill[fi]()
                        fi += 1
                        acc -= 1.0
                    if kind == 2:
                        for h in range(2):
                            nc.vector.tensor_add(st[:, h, :], st[:, h, :],
                                                 mb_t[a][:])
                    pt = pt_pool.tile([128, 2, QT], f16, tag="pt", name="pt")
                    nc.scalar.activation(out=pt[:, :, q0:QT], in_=st[:, :, q0:QT],
                                         func=Exp, scale=float(SCALE))
                    if kind == 1:
                        si, w = bopt
                        nc.vector.tensor_mul(
                            pt[:, :, q0:q0 + w], pt[:, :, q0:q0 + w],
                            mm_t[si][:, None, 0:w].broadcast_to([KTL, 2, w]))
                    for h in range(2):
                        nc.tensor.matmul(
                            ot_ps[h][:, q0:QT],
                            v_sb[kt][:, 2 * hp + h, :],
                            pt[:, h, q0:QT],
                            start=(ki == 0), stop=(ki == len(kts) - 1))
                # softmax normalization. All PSUM reads come first (2
                # reciprocals + 2 stage copies) so the ot banks free fast;
                # the 1/denom row broadcast runs on the idle GpSimd engine,
                # keeping the PE instruction stream free of norm work.
                r_rows, osbs = [], []
                for h in range(2):
                    r_row = rr_pool.tile([1, QT], f32, tag=f"rrow{h}",
                                         name="r_row")
                    nc.vector.reciprocal_approx_fast(out=r_row[:],
                                                     in_=ot_ps[h][0:1, :])
                    r_rows.append(r_row)
                    osb = rr_pool.tile([DK, QT], f16, tag=f"osb{h}",
                                       name="osb")
                    nc.vector.tensor_copy(out=osb[:], in_=ot_ps[h][DK:2 * DK, :])
                    osbs.append(osb)
                for h in range(2):
                    r16 = rr_pool.tile([1, QT], f16, tag=f"r16{h}", name="r16")
                    nc.vector.tensor_copy(out=r16[:], in_=r_rows[h][:])
                    rb16 = rr_pool.tile([DK, QT], f16, tag=f"rb{h}", name="rb16")
                    nc.gpsimd.partition_broadcast(rb16[:], r16[:], channels=DK)
                    nc.vector.tensor_mul(otq[hp][64 * h:64 * h + 64, :],
                                         osbs[h][:], rb16[:])
            while fi < len(fill):
                fill[fi]()
                fi += 1
            prev_otq = otq
            otq_hist[qi] = otq
        # final q-tile's output projection
        for piece in p3_pieces(N_QT - 1, prev_otq):
            piece()
    nc.compile()
    return nc


def kernel(encodings_for_qkv, mask, w_qkv, b_qkv, w_o):
    global last_results
    from concourse.bass_utils import run_bass_kernel_spmd

    x = np.ascontiguousarray(np.asarray(encodings_for_qkv, dtype=np.float32))
    mask2d = np.asarray(mask).reshape(S, S).astype(bool)
    w_qkv = np.asarray(w_qkv, dtype=np.float32)
    b_qkv = np.asarray(b_qkv, dtype=np.float32)
    w_o = np.asarray(w_o, dtype=np.float32)

    plan, strips, biases = _classify_mask(mask2d)
    key = repr([[e[:3] + ((e[3][0], e[3][1]) if e[3] else None,) for e in row]
                for row in plan]) + repr(sorted(
                    (k, v) for k, v in os.environ.items() if k.startswith("KERNEL_")))
    if key not in _cache:
        _cache[key] = _build(plan, len(strips), len(biases))
    nc = _cache[key]

    maskm = (np.stack(strips) if strips
             else np.zeros((1, KTL, STRW), dtype=np.float32))
    maskb = (np.stack(biases) if biases
             else np.zeros((1, KTL, QT), dtype=np.float32))
    wT = np.ascontiguousarray(w_qkv.T)        # [D, 3D]
    woT_full = w_o.T                          # [D(in), D(out)]

    in_maps = []
    for c in range(8):
        b, g = divmod(c, 2)
        cols = slice(GD * g, GD * (g + 1))
        w_qk_g = np.ascontiguousarray(
            np.concatenate([wT[:, 0 * D:][:, cols], wT[:, 1 * D:][:, cols]], axis=1))
        b_qk_g = np.ascontiguousarray(
            np.concatenate([b_qkv[0 * D:1 * D][cols], b_qkv[1 * D:2 * D][cols]]))
        w_v_g = np.ascontiguousarray(wT[:, 2 * D:][:, cols])
        wo_T_g = np.ascontiguousarray(woT_full[cols, :])
        in_maps.append({
            "xT": np.ascontiguousarray(x[b].T).astype(np.float16),
            "w_qk": w_qk_g.astype(np.float16), "b_qk": b_qk_g,
            "w_v": w_v_g.astype(np.float16),
            "wo_T": wo_T_g.astype(np.float16),
            "maskm": maskm.astype(np.float16), "maskb": maskb,
        })

    trace = bool(int(os.environ.get("KERNEL_PROFILE", "0")))
    res = run_bass_kernel_spmd(nc, in_maps, core_ids=list(range(8)),
                               trace=trace,
                               trace_cores=list(range(8)) if trace else None)
    last_results = res

    out = np.empty((B, S, D), dtype=np.float32)
    for b in range(B):
        acc = (res.results[2 * b]["outT"].astype(np.float32)
               + res.results[2 * b + 1]["outT"].astype(np.float32))
        out[b] = acc.T
    # V-bias epilogue: softmax rows sum to 1, so the V bias contributes a
    # constant (b_v @ w_o.T) to every sequence position.
    out += (b_qkv[2 * D:] @ woT_full).reshape(1, 1, D)
    return out



# revision 18
# speedup vs baseline: 1.1694x; 1.0191x over previous
"""Multi-head attention (B=4, S=2048, D=1024, H=16) on 8 TRN2 NeuronCores.

Sharding: core c -> (batch b = c//2, head-group g = c%2 of 8 heads).
Data parallel over batch, tensor parallel over heads; each core computes
its group's QKV projection slices, causal attention for its 8 heads, and
the partial output projection. Host sums the two per-batch partials
(the tensor-parallel unshard) and adds the V-bias epilogue.

On-device layout is "features on partitions": x, Q, K arrive/stay
transposed [feat, seq]; attention scores are computed directly in
transposed form S.T[k, q] so the exp'd probabilities feed the PV matmul
without any on-chip transpose. The softmax denominator rides the PV
matmul as an appended ones-column of V; normalization is a fast
reciprocal + K=1 broadcast matmul + DVE multiply.

Scheduling: TensorE executes in program order, so the QKV projection for
seq block sb+1 and the output projection for q-tile qi-1 are emitted as
small 2-PSUM-bank pieces woven between attention blocks of q-tile qi.
That keeps the PE busy while the (otherwise pacing) softmax EXP runs on
ScalarE, instead of running projection and attention as serial phases.
The first seq block's projection uses a transient 8-bank PSUM layout so
the first matmul only waits for one x/w chunk pair.
"""

import os
import numpy as np

B, S, D, H = 4, 2048, 1024, 16
DK = D // H          # 64
HPC = H // 2         # heads per core = 8
GD = HPC * DK        # group feature width = 512
QT = 512             # q-tile width (free dim of S.T chunks)
KTL = 128            # k-tile length (partition dim of S.T chunks)
N_QT = S // QT       # 4
N_KT = S // KTL      # 16
SB = 512             # projection seq block
STRW = 256           # padded width of multiplicative mask strips
NEG = np.float32(-1e9)
SCALE = 1.0 / np.sqrt(np.float32(DK))

_cache = {}
last_results = None


def _classify_mask(mask2d):
    """Classify each (q-tile, k-tile) block of the [S,S] bool mask.

    Returns (plan, strips, biases):
      plan[qi] = list over valid kt of (kt, kind, a, b):
        kind 0 = clean (no masking)
        kind 1 = staircase: a = q0 (suffix start), b = (strip_idx, strip_w)
        kind 2 = general:   a = bias_idx
      strips: list of [KTL, STRW] f32 0/1 multiplicative masks (padded)
      biases: list of [KTL, QT] f32 additive -1e9/0 masks
    Blocks are in S.T (k, q) layout.
    """
    kl = np.arange(KTL)[:, None]
    ql = np.arange(QT)[None, :]
    plan = []
    strips, strip_keys = [], {}
    biases, bias_keys = [], {}
    for qi in range(N_QT):
        row = []
        for kt in range(N_KT):
            blk = mask2d[qi * QT:(qi + 1) * QT, kt * KTL:(kt + 1) * KTL].T
            if blk.all():
                continue
            if not blk.any():
                row.append((kt, 0, 0, None))
                continue
            dj = kt * KTL - qi * QT
            stair = (0 <= dj <= QT - KTL and np.array_equal(blk, kl + dj > ql)
                     and not os.environ.get("KERNEL_NO_STAIR"))
            q0 = min(dj, QT - STRW) if stair else 0
            if stair and (q0 == 0 or row):
                w = min(dj + KTL, QT) - q0
                pat = (~blk[:, q0:q0 + w]).astype(np.float32)
                key = (w, pat.tobytes())
                if key not in strip_keys:
                    strip_keys[key] = len(strips)
                    p = np.zeros((KTL, STRW), np.float32)
                    p[:, :w] = pat
                    strips.append(p)
                row.append((kt, 1, q0, (strip_keys[key], w)))
            else:
                bias = np.where(blk, NEG, np.float32(0.0))
                key = bias.tobytes()
                if key not in bias_keys:
                    bias_keys[key] = len(biases)
                    biases.append(bias)
                row.append((kt, 2, bias_keys[key], None))
        if not row:
            # fully-masked q-row: include everything with full bias so the
            # softmax matches the reference's uniform distribution.
            bias = np.full((KTL, QT), NEG, np.float32)
            key = bias.tobytes()
            if key not in bias_keys:
                bias_keys[key] = len(biases)
                biases.append(bias)
            row = [(kt, 2, bias_keys[key], None) for kt in range(N_KT)]
        plan.append(row)
    return plan, strips, biases


def _build(plan, n_strips, n_biases):
    import concourse.bacc as bacc
    import concourse.tile as tile
    import concourse.mybir as mybir
    from contextlib import ExitStack

    f32 = mybir.dt.float32
    f16 = mybir.dt.float16
    Exp = mybir.ActivationFunctionType.Exp

    nc = bacc.Bacc(trn_type="TRN2", target_bir_lowering=False, debug=False)
    xT = nc.dram_tensor("xT", [D, S], f16, kind="ExternalInput").ap()
    w_qk = nc.dram_tensor("w_qk", [D, 2 * GD], f16, kind="ExternalInput").ap()
    b_qk = nc.dram_tensor("b_qk", [2 * GD], f32, kind="ExternalInput").ap()
    w_v = nc.dram_tensor("w_v", [D, GD], f16, kind="ExternalInput").ap()
    wo_T = nc.dram_tensor("wo_T", [GD, D], f16, kind="ExternalInput").ap()
    maskm = nc.dram_tensor("maskm", [max(n_strips, 1), KTL, STRW], f16,
                           kind="ExternalInput").ap()
    maskb = nc.dram_tensor("maskb", [max(n_biases, 1), KTL, QT], f32,
                           kind="ExternalInput").ap()
    outT = nc.dram_tensor("outT", [D, S], f16, kind="ExternalOutput").ap()

    ND = D // 128       # 8 contraction chunks
    NM = 2 * GD // 128  # 8 QK feature chunks (0-3 = Q.T, 4-7 = K.T)
    NK3 = GD // 128     # 4 output-projection contraction chunks
    NSB = S // SB       # 4

    with tile.TileContext(nc) as tc, ExitStack() as ctx:
        singles = ctx.enter_context(tc.tile_pool(name="singles", bufs=1))
        qkt_pool = ctx.enter_context(tc.tile_pool(name="qkt", bufs=1))
        v_pool = ctx.enter_context(tc.tile_pool(name="vp", bufs=1))
        otq_pool = ctx.enter_context(tc.tile_pool(name="otq", bufs=3))
        xs_pool = ctx.enter_context(tc.tile_pool(name="p1x", bufs=2))

        qkt = [qkt_pool.tile([128, S], f16, tag=f"qkt{m}", name=f"qkt{m}")
               for m in range(NM)]
        ones_col = singles.tile([1, DK], f16)
        v_sb = [v_pool.tile([128, HPC, 128], f16, tag=f"v{t}", name=f"v{t}")
                for t in range(N_KT)]
        bqk_t = singles.tile([128, NM], f32)
        wqk_t = [singles.tile([128, 2 * GD], f16, tag=f"wqk{k}", name=f"wqk{k}")
                 for k in range(ND)]
        wv_t = [singles.tile([128, GD], f16, tag=f"wv{k}", name=f"wv{k}")
                for k in range(ND)]
        wo_t = [singles.tile([128, D], f16, tag=f"wo{k}", name=f"wo{k}")
                for k in range(NK3)]
        mm_t = [singles.tile([KTL, STRW], f16, tag=f"mm{i}", name=f"mm{i}")
                for i in range(n_strips)]
        mb_t = [singles.tile([KTL, QT], f32, tag=f"mb{i}", name=f"mb{i}")
                for i in range(n_biases)]

        # Load order matters: the first seq block's projection contracts
        # k-outer, so (xs[k], wqk[k]) pairs in k order come first; wv/wo/
        # masks follow and finish while the QK matmuls run.
        xs_cur = [xs_pool.tile([128, SB], f16, tag=f"x{k}", name=f"xs{k}")
                  for k in range(ND)]
        for k in range(ND):
            nc.sync.dma_start(out=xs_cur[k], in_=xT[128 * k:128 * (k + 1), 0:SB])
            nc.sync.dma_start(out=wqk_t[k], in_=w_qk[128 * k:128 * (k + 1)])
        nc.sync.dma_start(out=bqk_t, in_=b_qk.rearrange("(m p) -> p m", p=128))
        for k in range(ND):
            nc.sync.dma_start(out=wv_t[k], in_=w_v[128 * k:128 * (k + 1)])
        for k in range(NK3):
            nc.sync.dma_start(out=wo_t[k], in_=wo_T[128 * k:128 * (k + 1)])
        for i in range(n_strips):
            nc.sync.dma_start(out=mm_t[i], in_=maskm[i])
        for i in range(n_biases):
            nc.sync.dma_start(out=mb_t[i], in_=maskb[i])
        nc.vector.memset(ones_col, 1.0)
        for t in range(N_KT):
            nc.vector.memset(v_sb[t][:, :, 0:1], 1.0)
            nc.vector.memset(v_sb[t][:, :, 1:DK], 0.0)

        # ==== projection for seq block 0: transient 8-bank k-outer layout
        # (first matmul only waits for xs[0] + wqk[0]) ====
        with tc.tile_pool(name="p1ps0", bufs=8, space="PSUM") as p1ps0:
            pss = [p1ps0.tile([128, SB], f32, tag="p10", name=f"ps0{m}")
                   for m in range(NM)]
            for k in range(ND):
                for m in range(NM):
                    nc.tensor.matmul(
                        pss[m][:], wqk_t[k][:, 128 * m:128 * (m + 1)],
                        xs_cur[k][:], start=(k == 0), stop=(k == ND - 1))
            for m in range(NM):
                nc.vector.tensor_scalar_add(qkt[m][:, 0:SB], pss[m][:],
                                            bqk_t[:, m:m + 1])
            for tt in range(SB // 128):
                ps = p1ps0.tile([128, GD], f32, tag="p10", name="ps0_v")
                for k in range(ND):
                    nc.tensor.matmul(
                        ps[:], xs_cur[k][:, 128 * tt:128 * (tt + 1)], wv_t[k][:],
                        start=(k == 0), stop=(k == ND - 1))
                nc.vector.tensor_copy(
                    out=v_sb[tt][:, :, DK:2 * DK],
                    in_=ps[:].rearrange("p (h d) -> p h d", h=HPC))

        # ==== steady-state pools: 4 + 2 + 2 = 8 PSUM banks ====
        st_pool = ctx.enter_context(tc.tile_pool(name="st", bufs=2, space="PSUM"))
        ot_pool = ctx.enter_context(tc.tile_pool(name="ot", bufs=2, space="PSUM"))
        p1p = ctx.enter_context(tc.tile_pool(name="p1p", bufs=2, space="PSUM"))
        pt_pool = ctx.enter_context(tc.tile_pool(name="pt", bufs=6))
        rr_pool = ctx.enter_context(tc.tile_pool(name="rr", bufs=4))
        p3o = ctx.enter_context(tc.tile_pool(name="p3o", bufs=4))

        xs_next = [None]

        def p1_pieces(sb):
            """Projection for seq block sb as 12 single-PSUM-chunk pieces
            (m-outer, k-inner contraction into 1 bank at a time)."""
            xs = xs_next[0]

            def qk_piece(m=0, xs=xs, sb=sb):
                ps = p1p.tile([128, SB], f32, tag="p1", name="ps_qk")
                for k in range(ND):
                    nc.tensor.matmul(
                        ps[:], wqk_t[k][:, 128 * m:128 * (m + 1)], xs[k][:],
                        start=(k == 0), stop=(k == ND - 1))
                nc.vector.tensor_scalar_add(
                    qkt[m][:, SB * sb:SB * (sb + 1)], ps[:], bqk_t[:, m:m + 1])

            def v_piece(tt=0, xs=xs, sb=sb):
                t = sb * (SB // 128) + tt
                ps = p1p.tile([128, GD], f32, tag="p1", name="ps_v")
                for k in range(ND):
                    nc.tensor.matmul(
                        ps[:], xs[k][:, 128 * tt:128 * (tt + 1)], wv_t[k][:],
                        start=(k == 0), stop=(k == ND - 1))
                nc.vector.tensor_copy(
                    out=v_sb[t][:, :, DK:2 * DK],
                    in_=ps[:].rearrange("p (h d) -> p h d", h=HPC))

            for m in range(NM):
                yield (lambda m=m: qk_piece(m))
            for tt in range(SB // 128):
                yield (lambda tt=tt: v_piece(tt))

        def p3_pieces(qi, otq):
            """Output projection for q-tile qi as 8 single-chunk pieces."""
            def piece(m=0, qi=qi, otq=otq):
                ps = p1p.tile([128, QT], f32, tag="p1", name="ps_o")
                for k in range(NK3):
                    nc.tensor.matmul(
                        ps[:], wo_t[k][:, 128 * m:128 * (m + 1)], otq[k][:],
                        start=(k == 0), stop=(k == NK3 - 1))
                ob = p3o.tile([128, QT], f16, tag="ob", name="ob")
                nc.vector.tensor_copy(out=ob[:], in_=ps[:])
                nc.sync.dma_start(
                    out=outT[128 * m:128 * (m + 1), QT * qi:QT * (qi + 1)],
                    in_=ob[:])
            for m in range(D // 128):
                yield (lambda m=m: piece(m))

        # ==== attention q-tiles with woven projection pieces ====
        prev_otq = None
        otq_hist = {}
        for qi in range(N_QT):
            fill = []
            if qi + 1 < NSB:
                xs_next[0] = [xs_pool.tile([128, SB], f16, tag=f"x{k}",
                                           name=f"xs{k}") for k in range(ND)]
                for k in range(ND):
                    nc.sync.dma_start(
                        out=xs_next[0][k],
                        in_=xT[128 * k:128 * (k + 1),
                               SB * (qi + 1):SB * (qi + 2)])
                fill.extend(p1_pieces(qi + 1))
            # out-projections are deferred toward the late q-tiles where
            # attention rows are long and weave fill is otherwise scarce:
            # qi2 runs p3(qi0); qi3 runs p3(qi1) and p3(qi2).
            if qi == 2:
                fill.extend(p3_pieces(0, otq_hist[0]))
            elif qi == 3:
                fill.extend(p3_pieces(1, otq_hist[1]))
                fill.extend(p3_pieces(2, otq_hist[2]))

            kts = plan[qi]
            n_blocks = max(len(kts) * (HPC // 2), 1)
            pace = len(fill) / n_blocks
            acc = 0.0
            fi = 0

            otq = [otq_pool.tile([128, QT], f16, tag=f"otq{m}", name=f"otq{m}")
                   for m in range(NK3)]
            for hp in range(HPC // 2):
                ot_ps = [ot_pool.tile([128, QT], f32, tag="ot", name="ot_ps")
                         for _ in range(2)]
                for ki, (kt, kind, a, bopt) in enumerate(kts):
                    q0 = a if kind == 1 else 0
                    st = st_pool.tile([128, 2, QT], f32, tag="st", name="st")
                    for h in range(2):
                        lo, hi = 64 * h, 64 * h + 64
                        nc.tensor.matmul(
                            st[:, h, q0:QT],
                            qkt[4 + hp][lo:hi, KTL * kt:KTL * (kt + 1)],
                            qkt[hp][lo:hi, QT * qi + q0:QT * (qi + 1)],
                            start=True, stop=True, tile_position=(64 * h, 0))
                    # weave projection pieces while EXP runs on ScalarE
                    acc += pace
                    while acc >= 1.0 and fi < len(fill):
                        fill[fi]()
                        fi += 1
                        acc -= 1.0
                    if kind == 2:
                        for h in range(2):
                            nc.vector.tensor_add(st[:, h, :], st[:, h, :],
                                                 mb_t[a][:])
                    pt = pt_pool.tile([128, 2, QT], f16, tag="pt", name="pt")
                    nc.scalar.activation(out=pt[:, :, q0:QT], in_=st[:, :, q0:QT],
                                         func=Exp, scale=float(SCALE))
                    if kind == 1:
                        si, w = bopt
                        nc.vector.tensor_mul(
                            pt[:, :, q0:q0 + w], pt[:, :, q0:q0 + w],
                            mm_t[si][:, None, 0:w].broadcast_to([KTL, 2, w]))
                    for h in range(2):
                        nc.tensor.matmul(
                            ot_ps[h][:, q0:QT],
                            v_sb[kt][:, 2 * hp + h, :],
                            pt[:, h, q0:QT],
                            start=(ki == 0), stop=(ki == len(kts) - 1))
                # softmax normalization. All PSUM reads come first (2
                # reciprocals + 2 stage copies) so the ot banks free fast;
                # the 1/denom row broadcast runs on the idle GpSimd engine,
                # keeping the PE instruction stream free of norm work.
                r_rows, osbs = [], []
                for h in range(2):
                    r_row = rr_pool.tile([1, QT], f32, tag=f"rrow{h}",
                                         name="r_row")
                    nc.vector.reciprocal_approx_fast(out=r_row[:],
                                                     in_=ot_ps[h][0:1, :])
                    r_rows.append(r_row)
                    osb = rr_pool.tile([DK, QT], f16, tag=f"osb{h}",
                                       name="osb")
                    nc.vector.tensor_copy(out=osb[:], in_=ot_ps[h][DK:2 * DK, :])
                    osbs.append(osb)
                for h in range(2):
                    rb32 = rr_pool.tile([DK, QT], f32, tag=f"rb{h}", name="rb32")
                    nc.gpsimd.partition_broadcast(rb32[:], r_rows[h][:],
                                                  channels=DK)
                    nc.vector.tensor_mul(otq[hp][64 * h:64 * h + 64, :],
                                         osbs[h][:], rb32[:])
            while fi < len(fill):
                fill[fi]()
                fi += 1
            prev_otq = otq
            otq_hist[qi] = otq
        # final q-tile's output projection
        for piece in p3_pieces(N_QT - 1, prev_otq):
            piece()
    nc.compile()
    return nc


def kernel(encodings_for_qkv, mask, w_qkv, b_qkv, w_o):
    global last_results
    from concourse.bass_utils import run_bass_kernel_spmd

    x = np.ascontiguousarray(np.asarray(encodings_for_qkv, dtype=np.float32))
    mask2d = np.asarray(mask).reshape(S, S).astype(bool)
    w_qkv = np.asarray(w_qkv, dtype=np.float32)
    b_qkv = np.asarray(b_qkv, dtype=np.float32)
    w_o = np.asarray(w_o, dtype=np.float32)

    plan, strips, biases = _classify_mask(mask2d)
    key = repr([[e[:3] + ((e[3][0], e[3][1]) if e[3] else None,) for e in row]
                for row in plan]) + repr(sorted(
                    (k, v) for k, v in os.environ.items() if k.startswith("KERNEL_")))
    if key not in _cache:
        _cache[key] = _build(plan, len(strips), len(biases))
    nc = _cache[key]

    maskm = (np.stack(strips) if strips
             else np.zeros((1, KTL, STRW), dtype=np.float32))
    maskb = (np.stack(biases) if biases
             else np.zeros((1, KTL, QT), dtype=np.float32))
    wT = np.ascontiguousarray(w_qkv.T)        # [D, 3D]
    woT_full = w_o.T                          # [D(in), D(out)]

    in_maps = []
    for c in range(8):
        b, g = divmod(c, 2)
        cols = slice(GD * g, GD * (g + 1))
        w_qk_g = np.ascontiguousarray(
            np.concatenate([wT[:, 0 * D:][:, cols], wT[:, 1 * D:][:, cols]], axis=1))
        b_qk_g = np.ascontiguousarray(
            np.concatenate([b_qkv[0 * D:1 * D][cols], b_qkv[1 * D:2 * D][cols]]))
        w_v_g = np.ascontiguousarray(wT[:, 2 * D:][:, cols])
        wo_T_g = np.ascontiguousarray(woT_full[cols, :])
        in_maps.append({
            "xT": np.ascontiguousarray(x[b].T).astype(np.float16),
            "w_qk": w_qk_g.astype(np.float16), "b_qk": b_qk_g,
            "w_v": w_v_g.astype(np.float16),
            "wo_T": wo_T_g.astype(np.float16),
            "maskm": maskm.astype(np.float16), "maskb": maskb,
        })

    trace = bool(int(os.environ.get("KERNEL_PROFILE", "0")))
    res = run_bass_kernel_spmd(nc, in_maps, core_ids=list(range(8)),
                               trace=trace,
                               trace_cores=list(range(8)) if trace else None)
    last_results = res

    out = np.empty((B, S, D), dtype=np.float32)
    for b in range(B):
        acc = (res.results[2 * b]["outT"].astype(np.float32)
               + res.results[2 * b + 1]["outT"].astype(np.float32))
        out[b] = acc.T
    # V-bias epilogue: softmax rows sum to 1, so the V bias contributes a
    # constant (b_v @ w_o.T) to every sequence position.
    out += (b_qkv[2 * D:] @ woT_full).reshape(1, 1, D)
    return out

